# revision 1
# baseline (speedup 1.0000x reference)
"""Trainium2 Bass kernel for nn_NNSDecoder (gnn_message_passing).

Reference computation (B=16, N=501, D=128, H=4):
    out[b,i,j] = fc3 . relu(fc2^T relu(feat @ fc1 + b1) + b2) + b3
    feat[b,i,j] = [cp_pre[b,i], cp_post[b,i], cd_pre[b,j], cd_post[b,j]]  (4H=16)

Key algebra: compat[b,n,h] = x[b,n] . (Wk[h] Wq[h]^T q_b), so every
pickup/delivery-side term is linear in h_hat / h_nb rows.  Folding the
head projections and fc1 together gives per-batch 128x32 maps:
    A[b] = h_hat[b] @ G_A1 + h_nb[b] @ G_A2          (N x 32, row/i term)
    C[b] = h_hat[b] @ G_C1 + h_nb[b] @ G_C2          (N x 32, col/j term)
    out[b,i,j] = w3 . relu(W2^T relu(A[b,i] + C[b,j] + b1) + b2) + b3

The tiny G matrices (and the index gathers defining them) are computed on
host; the device does all O(N) and O(N^2) work:
  - per batch: two matmul pairs produce A^T (32 x N) and C^T stacked 4x
    (128 x N) in PSUM; small copies build the SBUF layouts.
  - per 4-i tile: X = relu(C_rep + A_col) fused tensor_scalar (add+max),
    fc2 as one block-diag 128x128 bf16 matmul, Y = relu(Z + b2)
    (split between ScalarE and DVE), fc3 as column-tiled 128x4 matmuls
    packing 4 tiles into one PSUM bank, one copy out, grouped DMA.

Sharding: batch dim 16 -> 8 cores x 2 batches (data parallel, weights
replicated). Full inputs in, full output out.
"""

import numpy as np

B, N, D, H = 16, 501, 128, 4
NCORES = 8
BPC = B // NCORES  # batches per core
NP = 504  # padded N: even, multiple of 4, fits one PSUM bank (<=512 f32)
NT = NP // 4  # 126 i-tiles of 4 rows each

# engine splits (tuned from traces): which t%CYCLE slots go where
Y_CYCLE = 3
Y_ON_ACT = (0, 1)  # Y = relu(Z+b2): these slots on ScalarE, rest on DVE
X_CYCLE = 5
X_ON_GPS = (4,)  # X = relu(C+A): these slots on GpSimd, rest on DVE

_cache = {}


def _build_program():
    import concourse.bacc as bacc
    import concourse.mybir as mybir
    from concourse.tile import TileContext

    F32 = mybir.dt.float32
    BF16 = mybir.dt.bfloat16
    F32R = mybir.dt.float32r
    F16 = mybir.dt.float16
    nc = bacc.Bacc("TRN2", target_bir_lowering=False, debug=False, num_devices=1)

    hhT = nc.dram_tensor("hhT", [BPC, D, NP], F32, kind="ExternalInput")
    hnT = nc.dram_tensor("hnT", [BPC, D, NP], F32, kind="ExternalInput")
    g1a = nc.dram_tensor("g1a", [BPC, D, 32], F32, kind="ExternalInput")
    g2a = nc.dram_tensor("g2a", [BPC, D, 32], F32, kind="ExternalInput")
    g1c = nc.dram_tensor("g1c", [BPC, D, 128], F32, kind="ExternalInput")
    g2c = nc.dram_tensor("g2c", [BPC, D, 128], F32, kind="ExternalInput")
    w2d = nc.dram_tensor("w2d", [D, 128], F16, kind="ExternalInput")
    w3d = nc.dram_tensor("w3d", [D, 4], F16, kind="ExternalInput")
    b1r = nc.dram_tensor("b1r", [D, 1], F32, kind="ExternalInput")
    b2r = nc.dram_tensor("b2r", [D, 1], F32, kind="ExternalInput")
    out = nc.dram_tensor("out", [BPC, N, N], F32, kind="ExternalOutput")

    add = mybir.AluOpType.add
    amax = mybir.AluOpType.max
    Relu = mybir.ActivationFunctionType.Relu

    with TileContext(nc) as tc:
        with (
            tc.tile_pool(name="const", bufs=1) as cpool,
            tc.tile_pool(name="batch", bufs=2) as bpool,
            tc.tile_pool(name="x", bufs=6) as xpool,
            tc.tile_pool(name="y", bufs=6) as ypool,
            tc.tile_pool(name="o", bufs=3) as opool,
            tc.tile_pool(name="pz", bufs=4, space="PSUM") as pzpool,
            tc.tile_pool(name="po", bufs=2, space="PSUM") as popool,
            tc.tile_pool(name="pac", bufs=1, space="PSUM") as pacpool,
            tc.tile_pool(name="paa", bufs=1, space="PSUM") as paapool,
        ):
            w2t = cpool.tile([D, 128], F16)
            nc.sync.dma_start(w2t[:], w2d.ap()[:, :])
            w3t = cpool.tile([D, 4], F16)
            nc.sync.dma_start(w3t[:], w3d.ap()[:, :])
            b1t = cpool.tile([D, 1], F32)
            nc.sync.dma_start(b1t[:], b1r.ap()[:, :])
            b2t = cpool.tile([D, 1], F32)
            nc.sync.dma_start(b2t[:], b2r.ap()[:, :])

            for b in range(BPC):
                hh = bpool.tile([D, NP], F32, tag="hh")
                nc.sync.dma_start(hh[:], hhT.ap()[b, :, :])
                hn = bpool.tile([D, NP], F32, tag="hn")
                nc.sync.dma_start(hn[:], hnT.ap()[b, :, :])
                g1at = bpool.tile([D, 32], F32, tag="g1a")
                nc.sync.dma_start(g1at[:], g1a.ap()[b, :, :])
                g2at = bpool.tile([D, 32], F32, tag="g2a")
                nc.sync.dma_start(g2at[:], g2a.ap()[b, :, :])
                g1ct = bpool.tile([D, 128], F32, tag="g1c")
                nc.sync.dma_start(g1ct[:], g1c.ap()[b, :, :])
                g2ct = bpool.tile([D, 128], F32, tag="g2c")
                nc.sync.dma_start(g2ct[:], g2c.ap()[b, :, :])

                # A^T (32 x NP) and C^T replicated 4x on partitions (128 x NP)
                paa = paapool.tile([32, NP], F32)
                nc.tensor.matmul(paa[:], g1at[:], hh[:], start=True, stop=False)
                nc.tensor.matmul(paa[:], g2at[:], hn[:], start=False, stop=True)
                pac = pacpool.tile([D, NP], F32)
                nc.tensor.matmul(pac[:], g1ct[:], hh[:], start=True, stop=False)
                nc.tensor.matmul(pac[:], g2ct[:], hn[:], start=False, stop=True)

                # crep = C^T(rep4) + b1   (one copy, bias folded in)
                crep = bpool.tile([D, NP], F16, tag="crep")
                nc.vector.tensor_scalar_add(crep[:], pac[:], b1t[:, 0:1])

                # a4[32r+k, t] = A^T[k, 4t+r]  (i-tile column layout)
                a4 = bpool.tile([D, NT], F32, tag="a4")
                paa_r = paa[:, :].rearrange("p (t r) -> p r t", r=4)
                for r in range(4):
                    nc.scalar.copy(a4[32 * r : 32 * r + 32, :], paa_r[:, r, :])

                # main pair loop: groups of 4 i-tiles (16 rows of out each)
                for g in range(0, NT, 4):
                    gtiles = list(range(g, min(g + 4, NT)))
                    po = popool.tile([D, NP], F32)
                    ys = []
                    for t in gtiles:
                        x = xpool.tile([D, NP], F16)
                        xeng = (
                            nc.gpsimd if t % X_CYCLE in X_ON_GPS else nc.vector
                        )
                        xeng.tensor_scalar(
                            out=x[:],
                            in0=crep[:],
                            scalar1=a4[:, t : t + 1],
                            scalar2=0.0,
                            op0=add,
                            op1=amax,
                        )
                        pz = pzpool.tile([D, NP], F32)
                        nc.tensor.matmul(pz[:], w2t[:], x[:], start=True, stop=True)
                        y = ypool.tile([D, NP], F16)
                        if t % Y_CYCLE in Y_ON_ACT:
                            nc.scalar.activation(y[:], pz[:], Relu, bias=b2t[:, 0:1])
                        else:
                            nc.vector.tensor_scalar(
                                out=y[:],
                                in0=pz[:],
                                scalar1=b2t[:, 0:1],
                                scalar2=0.0,
                                op0=add,
                                op1=amax,
                            )
                        ys.append(y)
                    for u, y in enumerate(ys):
                        nc.tensor.matmul(
                            po[32 * u : 32 * u + 4, :],
                            w3t[:],
                            y[:],
                            start=True,
                            stop=True,
                            tile_position=(0, 32 * u),
                        )
                    ob = opool.tile([D, NP], F32)
                    nc.scalar.copy(ob[:], po[:])
                    for u, t in enumerate(gtiles):
                        i0 = 4 * t
                        ni = min(4, N - i0)
                        if ni > 0:
                            nc.sync.dma_start(
                                out.ap()[b, i0 : i0 + ni, :],
                                ob[32 * u : 32 * u + ni, 0:N],
                            )

    nc.compile()
    return nc


def _host_prep(h_hat, pos_pickup, pos_delivery, solution, Wq1, Wk1, Wq2, Wk2, fc1_w):
    """Per-batch tiny maps G (128x32 each) + transposed/padded node features."""
    f32 = np.float32
    h_hat = np.asarray(h_hat, f32)
    pp = np.asarray(pos_pickup).astype(np.int64)
    pd = np.asarray(pos_delivery).astype(np.int64)
    sol = np.asarray(solution).astype(np.int64)
    Wq1 = np.asarray(Wq1, f32)
    Wk1 = np.asarray(Wk1, f32)
    Wq2 = np.asarray(Wq2, f32)
    Wk2 = np.asarray(Wk2, f32)
    fc1_w = np.asarray(fc1_w, f32)

    hhT = np.zeros((B, D, NP), f32)
    hnT = np.zeros((B, D, NP), f32)
    g1a = np.zeros((B, D, 32), f32)
    g2a = np.zeros((B, D, 32), f32)
    g1c = np.zeros((B, D, 128), f32)
    g2c = np.zeros((B, D, 128), f32)

    for b in range(B):
        hb = h_hat[b]  # (N, D)
        hnb = hb[sol[b]]  # (N, D) gathered neighbours
        hhT[b, :, :N] = hb.T
        hnT[b, :, :N] = hnb.T
        p = hb[pp[b]]  # (D,)
        dv = hb[pd[b]]
        # u[h] = Wk[h] @ (Wq[h]^T @ q): compat[n,h] = x[n] . u[h]
        U1p = np.stack([Wk1[h] @ (Wq1[h].T @ p) for h in range(H)], axis=1)
        U2p = np.stack([Wk2[h] @ (Wq2[h].T @ p) for h in range(H)], axis=1)
        U1d = np.stack([Wk1[h] @ (Wq1[h].T @ dv) for h in range(H)], axis=1)
        U2d = np.stack([Wk2[h] @ (Wq2[h].T @ dv) for h in range(H)], axis=1)
        g1a[b] = U1p @ fc1_w[0:4]  # h_hat -> A
        g2a[b] = U2p @ fc1_w[4:8]  # h_nb  -> A
        gc1 = U1d @ fc1_w[8:12]  # h_hat -> C
        gc2 = U2d @ fc1_w[12:16]  # h_nb  -> C
        g1c[b] = np.tile(gc1, (1, 4))
        g2c[b] = np.tile(gc2, (1, 4))
    return hhT, hnT, g1a, g2a, g1c, g2c


_last_results = None


def kernel(
    h_hat,
    pos_pickup,
    pos_delivery,
    solution,
    Wq1,
    Wk1,
    Wq2,
    Wk2,
    fc1_w,
    fc1_b,
    fc2_w,
    fc2_b,
    fc3_w,
    fc3_b,
):
    global _last_results
    import ml_dtypes
    from concourse.bass_utils import run_bass_kernel_spmd

    f32 = np.float32
    bf16 = ml_dtypes.bfloat16
    fc2_w = np.asarray(fc2_w, f32)
    fc1_b = np.asarray(fc1_b, f32)
    fc2_b = np.asarray(fc2_b, f32)
    fc3_w = np.asarray(fc3_w, f32)
    fc3_b = np.asarray(fc3_b, f32)

    hhT, hnT, g1a, g2a, g1c, g2c = _host_prep(
        h_hat, pos_pickup, pos_delivery, solution, Wq1, Wk1, Wq2, Wk2,
        np.asarray(fc1_w, f32),
    )

    # block-diagonal packed MLP weights (4 independent 32-blocks)
    w2d = np.zeros((D, 128), f32)
    w3d = np.zeros((D, 4), f32)
    for r in range(4):
        w2d[32 * r : 32 * r + 32, 32 * r : 32 * r + 32] = fc2_w
        w3d[32 * r : 32 * r + 32, r : r + 1] = fc3_w.reshape(32, 1)
    b1r = np.tile(fc1_b.reshape(32, 1), (4, 1)).astype(f32)
    b2r = np.tile(fc2_b.reshape(32, 1), (4, 1)).astype(f32)

    if "nc" not in _cache:
        _cache["nc"] = _build_program()
    nc = _cache["nc"]

    in_maps = []
    for c in range(NCORES):
        bs = slice(BPC * c, BPC * (c + 1))
        in_maps.append(
            {
                "hhT": np.ascontiguousarray(hhT[bs]),
                "hnT": np.ascontiguousarray(hnT[bs]),
                "g1a": np.ascontiguousarray(g1a[bs]),
                "g2a": np.ascontiguousarray(g2a[bs]),
                "g1c": np.ascontiguousarray(g1c[bs]),
                "g2c": np.ascontiguousarray(g2c[bs]),
                "w2d": w2d.astype(np.float16),
                "w3d": w3d.astype(np.float16),
                "b1r": b1r,
                "b2r": b2r,
            }
        )

    res = run_bass_kernel_spmd(nc, in_maps, core_ids=list(range(NCORES)))
    _last_results = res

    out = np.concatenate([res.results[c]["out"] for c in range(NCORES)], axis=0)
    b3 = float(fc3_b.reshape(-1)[0])
    if b3 != 0.0:
        out = out + b3
    return out.astype(f32)



# revision 2
# speedup vs baseline: 2.8818x; 2.8818x over previous
"""Trainium2 Bass kernel for nn_NNSDecoder (gnn_message_passing).

Reference computation (B=16, N=501, D=128, H=4):
    out[b,i,j] = fc3 . relu(fc2^T relu(feat @ fc1 + b1) + b2) + b3
    feat[b,i,j] = [cp_pre[b,i], cp_post[b,i], cd_pre[b,j], cd_post[b,j]]  (4H=16)

Key algebra: compat[b,n,h] = x[b,n] . (Wk[h] Wq[h]^T q_b), so every
pickup/delivery-side term is linear in h_hat / h_nb rows.  Folding the
head projections and fc1 together gives per-batch 128x32 maps:
    A[b] = h_hat[b] @ G_A1 + h_nb[b] @ G_A2          (N x 32, row/i term)
    C[b] = h_hat[b] @ G_C1 + h_nb[b] @ G_C2          (N x 32, col/j term)
    out[b,i,j] = w3 . relu(W2^T relu(A[b,i] + C[b,j] + b1) + b2) + b3

The tiny G matrices (and the index gathers defining them) are computed on
host; the device does all O(N) and O(N^2) work:
  - per batch: two matmul pairs produce A^T (32 x N) and C^T stacked 4x
    (128 x N) in PSUM; small copies build the SBUF layouts.
  - per 4-i tile: X = relu(C_rep + A_col) fused tensor_scalar (add+max),
    fc2 as one block-diag 128x128 bf16 matmul, Y = relu(Z + b2)
    (split between ScalarE and DVE), fc3 as column-tiled 128x4 matmuls
    packing 4 tiles into one PSUM bank, one copy out, grouped DMA.

Sharding: batch dim 16 -> 8 cores x 2 batches (data parallel, weights
replicated). Full inputs in, full output out.
"""

import numpy as np

B, N, D, H = 16, 501, 128, 4
NCORES = 8
BPC = B // NCORES  # batches per core
NP = 504  # padded N: even, multiple of 4, fits one PSUM bank (<=512 f32)
NT = NP // 4  # 126 i-tiles of 4 rows each

# engine splits (tuned from traces): which t%CYCLE slots go where
Y_CYCLE = 3
Y_ON_ACT = (0, 1)  # Y = relu(Z+b2): these slots on ScalarE, rest on DVE
X_CYCLE = 5
X_ON_GPS = ()  # X = relu(C+A): GpSimd is 21x below spec AND starves DVE; keep all on DVE

_cache = {}


def _build_program():
    import concourse.bacc as bacc
    import concourse.mybir as mybir
    from concourse.tile import TileContext

    F32 = mybir.dt.float32
    BF16 = mybir.dt.bfloat16
    F32R = mybir.dt.float32r
    F16 = mybir.dt.float16
    nc = bacc.Bacc("TRN2", target_bir_lowering=False, debug=False, num_devices=1)

    hhT = nc.dram_tensor("hhT", [BPC, D, NP], F32, kind="ExternalInput")
    hnT = nc.dram_tensor("hnT", [BPC, D, NP], F32, kind="ExternalInput")
    g1a = nc.dram_tensor("g1a", [BPC, D, 32], F32, kind="ExternalInput")
    g2a = nc.dram_tensor("g2a", [BPC, D, 32], F32, kind="ExternalInput")
    g1c = nc.dram_tensor("g1c", [BPC, D, 128], F32, kind="ExternalInput")
    g2c = nc.dram_tensor("g2c", [BPC, D, 128], F32, kind="ExternalInput")
    w2d = nc.dram_tensor("w2d", [D, 128], F16, kind="ExternalInput")
    w3d = nc.dram_tensor("w3d", [D, 4], F16, kind="ExternalInput")
    b1r = nc.dram_tensor("b1r", [D, 1], F32, kind="ExternalInput")
    b2r = nc.dram_tensor("b2r", [D, 1], F32, kind="ExternalInput")
    out = nc.dram_tensor("out", [BPC, N, N], F32, kind="ExternalOutput")

    add = mybir.AluOpType.add
    amax = mybir.AluOpType.max
    Relu = mybir.ActivationFunctionType.Relu

    with TileContext(nc) as tc:
        with (
            tc.tile_pool(name="const", bufs=1) as cpool,
            tc.tile_pool(name="batch", bufs=2) as bpool,
            tc.tile_pool(name="x", bufs=6) as xpool,
            tc.tile_pool(name="y", bufs=6) as ypool,
            tc.tile_pool(name="o", bufs=3) as opool,
            tc.tile_pool(name="pz", bufs=4, space="PSUM") as pzpool,
            tc.tile_pool(name="po", bufs=2, space="PSUM") as popool,
            tc.tile_pool(name="pac", bufs=1, space="PSUM") as pacpool,
            tc.tile_pool(name="paa", bufs=1, space="PSUM") as paapool,
        ):
            w2t = cpool.tile([D, 128], F16)
            nc.sync.dma_start(w2t[:], w2d.ap()[:, :])
            w3t = cpool.tile([D, 4], F16)
            nc.sync.dma_start(w3t[:], w3d.ap()[:, :])
            b1t = cpool.tile([D, 1], F32)
            nc.sync.dma_start(b1t[:], b1r.ap()[:, :])
            b2t = cpool.tile([D, 1], F32)
            nc.sync.dma_start(b2t[:], b2r.ap()[:, :])

            for b in range(BPC):
                hh = bpool.tile([D, NP], F32, tag="hh")
                nc.sync.dma_start(hh[:], hhT.ap()[b, :, :])
                hn = bpool.tile([D, NP], F32, tag="hn")
                nc.sync.dma_start(hn[:], hnT.ap()[b, :, :])
                g1at = bpool.tile([D, 32], F32, tag="g1a")
                nc.sync.dma_start(g1at[:], g1a.ap()[b, :, :])
                g2at = bpool.tile([D, 32], F32, tag="g2a")
                nc.sync.dma_start(g2at[:], g2a.ap()[b, :, :])
                g1ct = bpool.tile([D, 128], F32, tag="g1c")
                nc.sync.dma_start(g1ct[:], g1c.ap()[b, :, :])
                g2ct = bpool.tile([D, 128], F32, tag="g2c")
                nc.sync.dma_start(g2ct[:], g2c.ap()[b, :, :])

                # A^T (32 x NP) and C^T replicated 4x on partitions (128 x NP)
                paa = paapool.tile([32, NP], F32)
                nc.tensor.matmul(paa[:], g1at[:], hh[:], start=True, stop=False)
                nc.tensor.matmul(paa[:], g2at[:], hn[:], start=False, stop=True)
                pac = pacpool.tile([D, NP], F32)
                nc.tensor.matmul(pac[:], g1ct[:], hh[:], start=True, stop=False)
                nc.tensor.matmul(pac[:], g2ct[:], hn[:], start=False, stop=True)

                # crep = C^T(rep4) + b1   (one copy, bias folded in)
                crep = bpool.tile([D, NP], F16, tag="crep")
                nc.vector.tensor_scalar_add(crep[:], pac[:], b1t[:, 0:1])

                # a4[32r+k, t] = A^T[k, 4t+r]  (i-tile column layout)
                a4 = bpool.tile([D, NT], F32, tag="a4")
                paa_r = paa[:, :].rearrange("p (t r) -> p r t", r=4)
                for r in range(4):
                    nc.scalar.copy(a4[32 * r : 32 * r + 32, :], paa_r[:, r, :])

                # main pair loop: groups of 4 i-tiles (16 rows of out each)
                for g in range(0, NT, 4):
                    gtiles = list(range(g, min(g + 4, NT)))
                    po = popool.tile([D, NP], F32)
                    ys = []
                    for t in gtiles:
                        x = xpool.tile([D, NP], F16)
                        xeng = (
                            nc.gpsimd if t % X_CYCLE in X_ON_GPS else nc.vector
                        )
                        xeng.tensor_scalar(
                            out=x[:],
                            in0=crep[:],
                            scalar1=a4[:, t : t + 1],
                            scalar2=0.0,
                            op0=add,
                            op1=amax,
                        )
                        pz = pzpool.tile([D, NP], F32)
                        nc.tensor.matmul(pz[:], w2t[:], x[:], start=True, stop=True)
                        y = ypool.tile([D, NP], F16)
                        if t % Y_CYCLE in Y_ON_ACT:
                            nc.scalar.activation(y[:], pz[:], Relu, bias=b2t[:, 0:1])
                        else:
                            nc.vector.tensor_scalar(
                                out=y[:],
                                in0=pz[:],
                                scalar1=b2t[:, 0:1],
                                scalar2=0.0,
                                op0=add,
                                op1=amax,
                            )
                        ys.append(y)
                    for u, y in enumerate(ys):
                        nc.tensor.matmul(
                            po[32 * u : 32 * u + 4, :],
                            w3t[:],
                            y[:],
                            start=True,
                            stop=True,
                            tile_position=(0, 32 * u),
                        )
                    ob = opool.tile([D, NP], F32)
                    nc.scalar.copy(ob[:], po[:])
                    for u, t in enumerate(gtiles):
                        i0 = 4 * t
                        ni = min(4, N - i0)
                        if ni > 0:
                            nc.sync.dma_start(
                                out.ap()[b, i0 : i0 + ni, :],
                                ob[32 * u : 32 * u + ni, 0:N],
                            )

    nc.compile()
    return nc


def _host_prep(h_hat, pos_pickup, pos_delivery, solution, Wq1, Wk1, Wq2, Wk2, fc1_w):
    """Per-batch tiny maps G (128x32 each) + transposed/padded node features."""
    f32 = np.float32
    h_hat = np.asarray(h_hat, f32)
    pp = np.asarray(pos_pickup).astype(np.int64)
    pd = np.asarray(pos_delivery).astype(np.int64)
    sol = np.asarray(solution).astype(np.int64)
    Wq1 = np.asarray(Wq1, f32)
    Wk1 = np.asarray(Wk1, f32)
    Wq2 = np.asarray(Wq2, f32)
    Wk2 = np.asarray(Wk2, f32)
    fc1_w = np.asarray(fc1_w, f32)

    hhT = np.zeros((B, D, NP), f32)
    hnT = np.zeros((B, D, NP), f32)
    g1a = np.zeros((B, D, 32), f32)
    g2a = np.zeros((B, D, 32), f32)
    g1c = np.zeros((B, D, 128), f32)
    g2c = np.zeros((B, D, 128), f32)

    for b in range(B):
        hb = h_hat[b]  # (N, D)
        hnb = hb[sol[b]]  # (N, D) gathered neighbours
        hhT[b, :, :N] = hb.T
        hnT[b, :, :N] = hnb.T
        p = hb[pp[b]]  # (D,)
        dv = hb[pd[b]]
        # u[h] = Wk[h] @ (Wq[h]^T @ q): compat[n,h] = x[n] . u[h]
        U1p = np.stack([Wk1[h] @ (Wq1[h].T @ p) for h in range(H)], axis=1)
        U2p = np.stack([Wk2[h] @ (Wq2[h].T @ p) for h in range(H)], axis=1)
        U1d = np.stack([Wk1[h] @ (Wq1[h].T @ dv) for h in range(H)], axis=1)
        U2d = np.stack([Wk2[h] @ (Wq2[h].T @ dv) for h in range(H)], axis=1)
        g1a[b] = U1p @ fc1_w[0:4]  # h_hat -> A
        g2a[b] = U2p @ fc1_w[4:8]  # h_nb  -> A
        gc1 = U1d @ fc1_w[8:12]  # h_hat -> C
        gc2 = U2d @ fc1_w[12:16]  # h_nb  -> C
        g1c[b] = np.tile(gc1, (1, 4))
        g2c[b] = np.tile(gc2, (1, 4))
    return hhT, hnT, g1a, g2a, g1c, g2c


_last_results = None


def kernel(
    h_hat,
    pos_pickup,
    pos_delivery,
    solution,
    Wq1,
    Wk1,
    Wq2,
    Wk2,
    fc1_w,
    fc1_b,
    fc2_w,
    fc2_b,
    fc3_w,
    fc3_b,
):
    global _last_results
    import ml_dtypes
    from concourse.bass_utils import run_bass_kernel_spmd

    f32 = np.float32
    bf16 = ml_dtypes.bfloat16
    fc2_w = np.asarray(fc2_w, f32)
    fc1_b = np.asarray(fc1_b, f32)
    fc2_b = np.asarray(fc2_b, f32)
    fc3_w = np.asarray(fc3_w, f32)
    fc3_b = np.asarray(fc3_b, f32)

    hhT, hnT, g1a, g2a, g1c, g2c = _host_prep(
        h_hat, pos_pickup, pos_delivery, solution, Wq1, Wk1, Wq2, Wk2,
        np.asarray(fc1_w, f32),
    )

    # block-diagonal packed MLP weights (4 independent 32-blocks)
    w2d = np.zeros((D, 128), f32)
    w3d = np.zeros((D, 4), f32)
    for r in range(4):
        w2d[32 * r : 32 * r + 32, 32 * r : 32 * r + 32] = fc2_w
        w3d[32 * r : 32 * r + 32, r : r + 1] = fc3_w.reshape(32, 1)
    b1r = np.tile(fc1_b.reshape(32, 1), (4, 1)).astype(f32)
    b2r = np.tile(fc2_b.reshape(32, 1), (4, 1)).astype(f32)

    if "nc" not in _cache:
        _cache["nc"] = _build_program()
    nc = _cache["nc"]

    in_maps = []
    for c in range(NCORES):
        bs = slice(BPC * c, BPC * (c + 1))
        in_maps.append(
            {
                "hhT": np.ascontiguousarray(hhT[bs]),
                "hnT": np.ascontiguousarray(hnT[bs]),
                "g1a": np.ascontiguousarray(g1a[bs]),
                "g2a": np.ascontiguousarray(g2a[bs]),
                "g1c": np.ascontiguousarray(g1c[bs]),
                "g2c": np.ascontiguousarray(g2c[bs]),
                "w2d": w2d.astype(np.float16),
                "w3d": w3d.astype(np.float16),
                "b1r": b1r,
                "b2r": b2r,
            }
        )

    res = run_bass_kernel_spmd(nc, in_maps, core_ids=list(range(NCORES)))
    _last_results = res

    out = np.concatenate([res.results[c]["out"] for c in range(NCORES)], axis=0)
    b3 = float(fc3_b.reshape(-1)[0])
    if b3 != 0.0:
        out = out + b3
    return out.astype(f32)



# revision 5
# speedup vs baseline: 3.3907x; 1.1766x over previous
"""Trainium2 Bass kernel for nn_NNSDecoder (gnn_message_passing).

Reference computation (B=16, N=501, D=128, H=4):
    out[b,i,j] = fc3 . relu(fc2^T relu(feat @ fc1 + b1) + b2) + b3
    feat[b,i,j] = [cp_pre[b,i], cp_post[b,i], cd_pre[b,j], cd_post[b,j]]  (4H=16)

Key algebra: compat[b,n,h] = x[b,n] . (Wk[h] Wq[h]^T q_b), so every
pickup/delivery-side term is linear in h_hat / h_nb rows.  Folding the
head projections and fc1 together gives per-batch 128x32 maps:
    A[b] = h_hat[b] @ G_A1 + h_nb[b] @ G_A2          (N x 32, row/i term)
    C[b] = h_hat[b] @ G_C1 + h_nb[b] @ G_C2          (N x 32, col/j term)
    out[b,i,j] = w3 . relu(W2^T relu(A[b,i] + C[b,j] + b1) + b2) + b3

The tiny G matrices (and the index gathers defining them) are computed on
host; the device does all O(N) and O(N^2) work.  Per 4-row i-tile t:
    X_t = relu(crep + a4[:,t])            (DVE tensor_scalar, f16, 2x mode)
    Z_t = W2blk @ X_t                     (PE matmul, block-diag f16)
    Y_t = relu(Z_t + b2)                  (ScalarE ACT / DVE, f16)
    po  = w3blk @ Y_t                     (PE matmul into packed PSUM)
i-tiles are processed in PAIRS sharing one 2-bank PSUM tile [128,1024]
(NP=512 = exact bank) so Y and the PSUM->SBUF copy run as single wide
ops; fc3 matmuls are emitted 2 pairs late (software pipelining) so the
PE never stalls waiting for Y; 8-tile supergroups drain with one wide
copy + 2 batched 16-row DMAs (partition-strided APs).

Sharding: batch dim 16 -> 8 cores x 2 batches (data parallel, weights
replicated). Full inputs in, full output out.
"""

import numpy as np

B, N, D, H = 16, 501, 128, 4
NCORES = 8
BPC = B // NCORES  # batches per core
NP = 512  # padded j: exact PSUM bank (512 f32 = 2KB)
NT = 126  # i-tiles of 4 rows (126*4 = 504 >= 501)
PAIRS = NT // 2  # 63 i-tile pairs per batch

# engine split knob: every Y2_DVE_EVERY-th pair's Y runs on DVE, rest ScalarE
Y2_DVE_EVERY = 4

_cache = {}


def _build_program():
    import concourse.bacc as bacc
    import concourse.mybir as mybir
    from concourse.tile import TileContext

    F32 = mybir.dt.float32
    F32R = mybir.dt.float32r
    F16 = mybir.dt.float16
    nc = bacc.Bacc("TRN2", target_bir_lowering=False, debug=False, num_devices=1)

    hhT = nc.dram_tensor("hhT", [BPC, D, NP], F16, kind="ExternalInput")
    hnT = nc.dram_tensor("hnT", [BPC, D, NP], F16, kind="ExternalInput")
    g1a = nc.dram_tensor("g1a", [BPC, D, 32], F16, kind="ExternalInput")
    g2a = nc.dram_tensor("g2a", [BPC, D, 32], F16, kind="ExternalInput")
    g1c = nc.dram_tensor("g1c", [BPC, D, 128], F16, kind="ExternalInput")
    g2c = nc.dram_tensor("g2c", [BPC, D, 128], F16, kind="ExternalInput")
    w2d = nc.dram_tensor("w2d", [D, 128], F16, kind="ExternalInput")
    w3d = nc.dram_tensor("w3d", [D, 4], F16, kind="ExternalInput")
    b1r = nc.dram_tensor("b1r", [D, 1], F32, kind="ExternalInput")
    b2r = nc.dram_tensor("b2r", [D, 1], F32, kind="ExternalInput")
    NSG = 16  # supergroups per batch (8 i-tiles / 32 rows each)
    raw = nc.dram_tensor("raw", [BPC, NSG, 100, 2 * NP], F32, kind="ExternalOutput")

    add = mybir.AluOpType.add
    amax = mybir.AluOpType.max
    Relu = mybir.ActivationFunctionType.Relu

    with TileContext(nc) as tc:
        with (
            tc.tile_pool(name="const", bufs=1) as cpool,
            tc.tile_pool(name="batch", bufs=2) as bpool,
            tc.tile_pool(name="x", bufs=8) as xpool,
            tc.tile_pool(name="y", bufs=6) as ypool,
            tc.tile_pool(name="o", bufs=3) as opool,
            tc.tile_pool(name="pz", bufs=2, space="PSUM") as pzpool,
            tc.tile_pool(name="po", bufs=2, space="PSUM") as popool,
        ):
            w2t = cpool.tile([D, 128], F16)
            nc.sync.dma_start(w2t[:], w2d.ap()[:, :])
            w3t = cpool.tile([D, 4], F16)
            nc.sync.dma_start(w3t[:], w3d.ap()[:, :])
            b1t = cpool.tile([D, 1], F32)
            nc.sync.dma_start(b1t[:], b1r.ap()[:, :])
            b2t = cpool.tile([D, 1], F32)
            nc.sync.dma_start(b2t[:], b2r.ap()[:, :])

            for b in range(BPC):
                hh = bpool.tile([D, NP], F16, tag="hh")
                nc.sync.dma_start(hh[:], hhT.ap()[b, :, :])
                hn = bpool.tile([D, NP], F16, tag="hn")
                nc.sync.dma_start(hn[:], hnT.ap()[b, :, :])
                g1at = bpool.tile([D, 32], F16, tag="g1a")
                nc.sync.dma_start(g1at[:], g1a.ap()[b, :, :])
                g2at = bpool.tile([D, 32], F16, tag="g2a")
                nc.sync.dma_start(g2at[:], g2a.ap()[b, :, :])
                g1ct = bpool.tile([D, 128], F16, tag="g1c")
                nc.sync.dma_start(g1ct[:], g1c.ap()[b, :, :])
                g2ct = bpool.tile([D, 128], F16, tag="g2c")
                nc.sync.dma_start(g2ct[:], g2c.ap()[b, :, :])

                # C^T replicated 4x on partitions (128 x NP) and A^T (32 x NP),
                # in PSUM tiles riding the pz ring.
                pac = pzpool.tile([D, 2 * NP], F32, tag="pz")
                nc.tensor.matmul(pac[:, 0:NP], g1ct[:], hh[:], start=True, stop=False)
                nc.tensor.matmul(pac[:, 0:NP], g2ct[:], hn[:], start=False, stop=True)
                paa = pzpool.tile([D, 2 * NP], F32, tag="pz")
                nc.tensor.matmul(
                    paa[0:32, 0:NP], g1at[:], hh[:], start=True, stop=False
                )
                nc.tensor.matmul(
                    paa[0:32, 0:NP], g2at[:], hn[:], start=False, stop=True
                )

                # crep = C^T(rep4) + b1   (bias folded in)
                crep = bpool.tile([D, NP], F16, tag="crep")
                nc.vector.tensor_scalar_add(crep[:], pac[:, 0:NP], b1t[:, 0:1])

                # a4[32r+k, t] = A^T[k, 4t+r]  (i-tile column layout)
                a4 = bpool.tile([D, NT], F32, tag="a4")
                paa_r = paa[0:32, 0 : 4 * NT].rearrange("p (t r) -> p r t", r=4)
                for r in range(4):
                    nc.scalar.copy(a4[32 * r : 32 * r + 32, :], paa_r[:, r, :])

                # main pair loop, software-pipelined:
                #   pair p:  X,X + fc2 pair matmuls
                #   pair p-1: Y2 wide op
                #   pair p-2: fc3 matmuls (+ copy/DMA when a supergroup fills)
                pend_pz = {}
                pend_y = {}
                po2 = None
                for p in range(PAIRS + 2):
                    if p < PAIRS:
                        t0 = 2 * p
                        xs = []
                        for t in (t0, t0 + 1):
                            x = xpool.tile([D, NP], F16, tag="x")
                            nc.vector.tensor_scalar(
                                out=x[:],
                                in0=crep[:],
                                scalar1=a4[:, t : t + 1],
                                scalar2=0.0,
                                op0=add,
                                op1=amax,
                            )
                            xs.append(x)
                        pz2 = pzpool.tile([D, 2 * NP], F32, tag="pz")
                        nc.tensor.matmul(
                            pz2[:, 0:NP], w2t[:], xs[0][:], start=True, stop=True
                        )
                        nc.tensor.matmul(
                            pz2[:, NP : 2 * NP], w2t[:], xs[1][:], start=True, stop=True
                        )
                        pend_pz[p] = pz2

                    py = p - 1
                    if 0 <= py < PAIRS:
                        pz2 = pend_pz.pop(py)
                        y2 = ypool.tile([D, 2 * NP], F16, tag="y2")
                        if py % Y2_DVE_EVERY == Y2_DVE_EVERY - 1:
                            nc.vector.tensor_scalar(
                                out=y2[:],
                                in0=pz2[:],
                                scalar1=b2t[:, 0:1],
                                scalar2=0.0,
                                op0=add,
                                op1=amax,
                            )
                        else:
                            nc.scalar.activation(y2[:], pz2[:], Relu, bias=b2t[:, 0:1])
                        pend_y[py] = y2

                    pf = p - 2
                    if 0 <= pf < PAIRS:
                        s, q = divmod(pf, 4)
                        if q == 0:
                            po2 = popool.tile([D, 2 * NP], F32, tag="po")
                        y2 = pend_y.pop(pf)
                        for j in (0, 1):
                            st = 2 * q + j
                            h, v = st // 4, st % 4
                            nc.tensor.matmul(
                                po2[32 * v : 32 * v + 4, h * NP : h * NP + NP],
                                w3t[:],
                                y2[:, j * NP : j * NP + NP],
                                start=True,
                                stop=True,
                                tile_position=(0, 32 * v),
                            )
                        if q == 3 or pf == PAIRS - 1:
                            # used fc3 partitions are {32v+r, r<4} (v = slot%4,
                            # max 99); dump them raw, host un-permutes rows.
                            ob2 = opool.tile([D, 2 * NP], F32, tag="ob")
                            nc.scalar.copy(ob2[0:100, :], po2[0:100, :])
                            nc.sync.dma_start(raw.ap()[b, s, :, :], ob2[0:100, :])

    nc.compile()
    return nc


def _host_prep(h_hat, pos_pickup, pos_delivery, solution, Wq1, Wk1, Wq2, Wk2, fc1_w):
    """Per-batch tiny maps G (128x32 each) + transposed/padded node features."""
    f32 = np.float32
    h_hat = np.asarray(h_hat, f32)
    pp = np.asarray(pos_pickup).astype(np.int64)
    pd = np.asarray(pos_delivery).astype(np.int64)
    sol = np.asarray(solution).astype(np.int64)
    Wq1 = np.asarray(Wq1, f32)
    Wk1 = np.asarray(Wk1, f32)
    Wq2 = np.asarray(Wq2, f32)
    Wk2 = np.asarray(Wk2, f32)
    fc1_w = np.asarray(fc1_w, f32)

    hhT = np.zeros((B, D, NP), f32)
    hnT = np.zeros((B, D, NP), f32)
    g1a = np.zeros((B, D, 32), f32)
    g2a = np.zeros((B, D, 32), f32)
    g1c = np.zeros((B, D, 128), f32)
    g2c = np.zeros((B, D, 128), f32)

    for b in range(B):
        hb = h_hat[b]  # (N, D)
        hnb = hb[sol[b]]  # (N, D) gathered neighbours
        hhT[b, :, :N] = hb.T
        hnT[b, :, :N] = hnb.T
        p = hb[pp[b]]  # (D,)
        dv = hb[pd[b]]
        # u[h] = Wk[h] @ (Wq[h]^T @ q): compat[n,h] = x[n] . u[h]
        U1p = np.stack([Wk1[h] @ (Wq1[h].T @ p) for h in range(H)], axis=1)
        U2p = np.stack([Wk2[h] @ (Wq2[h].T @ p) for h in range(H)], axis=1)
        U1d = np.stack([Wk1[h] @ (Wq1[h].T @ dv) for h in range(H)], axis=1)
        U2d = np.stack([Wk2[h] @ (Wq2[h].T @ dv) for h in range(H)], axis=1)
        g1a[b] = U1p @ fc1_w[0:4]  # h_hat -> A
        g2a[b] = U2p @ fc1_w[4:8]  # h_nb  -> A
        gc1 = U1d @ fc1_w[8:12]  # h_hat -> C
        gc2 = U2d @ fc1_w[12:16]  # h_nb  -> C
        g1c[b] = np.tile(gc1, (1, 4))
        g2c[b] = np.tile(gc2, (1, 4))
    return hhT, hnT, g1a, g2a, g1c, g2c


_last_results = None


def kernel(
    h_hat,
    pos_pickup,
    pos_delivery,
    solution,
    Wq1,
    Wk1,
    Wq2,
    Wk2,
    fc1_w,
    fc1_b,
    fc2_w,
    fc2_b,
    fc3_w,
    fc3_b,
):
    global _last_results
    from concourse.bass_utils import run_bass_kernel_spmd

    f32 = np.float32
    fc2_w = np.asarray(fc2_w, f32)
    fc1_b = np.asarray(fc1_b, f32)
    fc2_b = np.asarray(fc2_b, f32)
    fc3_w = np.asarray(fc3_w, f32)
    fc3_b = np.asarray(fc3_b, f32)

    hhT, hnT, g1a, g2a, g1c, g2c = _host_prep(
        h_hat, pos_pickup, pos_delivery, solution, Wq1, Wk1, Wq2, Wk2,
        np.asarray(fc1_w, f32),
    )

    # block-diagonal packed MLP weights (4 independent 32-blocks)
    w2d = np.zeros((D, 128), f32)
    w3d = np.zeros((D, 4), f32)
    for r in range(4):
        w2d[32 * r : 32 * r + 32, 32 * r : 32 * r + 32] = fc2_w
        w3d[32 * r : 32 * r + 32, r : r + 1] = fc3_w.reshape(32, 1)
    b1r = np.tile(fc1_b.reshape(32, 1), (4, 1)).astype(f32)
    b2r = np.tile(fc2_b.reshape(32, 1), (4, 1)).astype(f32)

    if "nc" not in _cache:
        _cache["nc"] = _build_program()
    nc = _cache["nc"]

    in_maps = []
    for c in range(NCORES):
        bs = slice(BPC * c, BPC * (c + 1))
        in_maps.append(
            {
                "hhT": np.ascontiguousarray(hhT[bs]).astype(np.float16),
                "hnT": np.ascontiguousarray(hnT[bs]).astype(np.float16),
                "g1a": np.ascontiguousarray(g1a[bs]).astype(np.float16),
                "g2a": np.ascontiguousarray(g2a[bs]).astype(np.float16),
                "g1c": np.ascontiguousarray(g1c[bs]).astype(np.float16),
                "g2c": np.ascontiguousarray(g2c[bs]).astype(np.float16),
                "w2d": w2d.astype(np.float16),
                "w3d": w3d.astype(np.float16),
                "b1r": b1r,
                "b2r": b2r,
            }
        )

    res = run_bass_kernel_spmd(nc, in_maps, core_ids=list(range(NCORES)))
    _last_results = res

    # un-permute: raw[b, s, 32v+r, 512h+j] holds out row 32s+16h+4v+r
    rows = np.arange(N)
    s_i = rows // 32
    rem = rows % 32
    h_i = rem // 16
    rem2 = rem % 16
    part = 32 * (rem2 // 4) + (rem2 % 4)
    foff = NP * h_i
    cols = np.arange(N)
    out = np.empty((B, N, N), f32)
    for c in range(NCORES):
        rawc = res.results[c]["raw"]  # [BPC, NSG, 100, 2*NP]
        for bb in range(BPC):
            out[BPC * c + bb] = rawc[bb, s_i[:, None], part[:, None], foff[:, None] + cols[None, :]]
    b3 = float(fc3_b.reshape(-1)[0])
    if b3 != 0.0:
        out = out + b3
    return out.astype(f32)


# revision 6
# speedup vs baseline: 3.3941x; 1.0010x over previous
"""Trainium2 Bass kernel for nn_NNSDecoder (gnn_message_passing).

Reference computation (B=16, N=501, D=128, H=4):
    out[b,i,j] = fc3 . relu(fc2^T relu(feat @ fc1 + b1) + b2) + b3
    feat[b,i,j] = [cp_pre[b,i], cp_post[b,i], cd_pre[b,j], cd_post[b,j]]  (4H=16)

Key algebra: compat[b,n,h] = x[b,n] . (Wk[h] Wq[h]^T q_b), so every
pickup/delivery-side term is linear in h_hat / h_nb rows.  Folding the
head projections and fc1 together gives per-batch 128x32 maps:
    A[b] = h_hat[b] @ G_A1 + h_nb[b] @ G_A2          (N x 32, row/i term)
    C[b] = h_hat[b] @ G_C1 + h_nb[b] @ G_C2          (N x 32, col/j term)
    out[b,i,j] = w3 . relu(W2^T relu(A[b,i] + C[b,j] + b1) + b2) + b3

The tiny G matrices (and the index gathers defining them) are computed on
host; the device does all O(N) and O(N^2) work.  Per 4-row i-tile t:
    X_t = relu(crep + a4[:,t])            (DVE tensor_scalar, f16, 2x mode)
    Z_t = W2blk @ X_t                     (PE matmul, block-diag f16)
    Y_t = relu(Z_t + b2)                  (ScalarE ACT / DVE, f16)
    po  = w3blk @ Y_t                     (PE matmul into packed PSUM)
i-tiles are processed in PAIRS sharing one 2-bank PSUM tile [128,1024]
(NP=512 = exact bank) so Y and the PSUM->SBUF copy run as single wide
ops; fc3 matmuls are emitted 2 pairs late (software pipelining) so the
PE never stalls waiting for Y; 8-tile supergroups drain with one wide
copy + 2 batched 16-row DMAs (partition-strided APs).

Sharding: batch dim 16 -> 8 cores x 2 batches (data parallel, weights
replicated). Full inputs in, full output out.
"""

import numpy as np

B, N, D, H = 16, 501, 128, 4
NCORES = 8
BPC = B // NCORES  # batches per core
NP = 512  # padded j: exact PSUM bank (512 f32 = 2KB)
NT = 126  # i-tiles of 4 rows (126*4 = 504 >= 501)
PAIRS = NT // 2  # 63 i-tile pairs per batch

# engine split knob: every Y2_DVE_EVERY-th pair's Y runs on DVE, rest ScalarE
Y2_DVE_EVERY = 4

_cache = {}


def _build_program():
    import concourse.bacc as bacc
    import concourse.mybir as mybir
    from concourse.tile import TileContext

    F32 = mybir.dt.float32
    F32R = mybir.dt.float32r
    F16 = mybir.dt.float16
    nc = bacc.Bacc("TRN2", target_bir_lowering=False, debug=False, num_devices=1)

    hhT = nc.dram_tensor("hhT", [BPC, D, NP], F16, kind="ExternalInput")
    hnT = nc.dram_tensor("hnT", [BPC, D, NP], F16, kind="ExternalInput")
    g1a = nc.dram_tensor("g1a", [BPC, D, 32], F16, kind="ExternalInput")
    g2a = nc.dram_tensor("g2a", [BPC, D, 32], F16, kind="ExternalInput")
    g1c = nc.dram_tensor("g1c", [BPC, D, 128], F16, kind="ExternalInput")
    g2c = nc.dram_tensor("g2c", [BPC, D, 128], F16, kind="ExternalInput")
    w2d = nc.dram_tensor("w2d", [D, 128], F16, kind="ExternalInput")
    w3d = nc.dram_tensor("w3d", [D, 4], F16, kind="ExternalInput")
    b1r = nc.dram_tensor("b1r", [D, 1], F32, kind="ExternalInput")
    b2r = nc.dram_tensor("b2r", [D, 1], F32, kind="ExternalInput")
    NSG = 16  # supergroups per batch (8 i-tiles / 32 rows each)
    raw = nc.dram_tensor("raw", [BPC, NSG, 100, 2 * NP], F32, kind="ExternalOutput")

    add = mybir.AluOpType.add
    amax = mybir.AluOpType.max
    Relu = mybir.ActivationFunctionType.Relu

    with TileContext(nc) as tc:
        with (
            tc.tile_pool(name="const", bufs=1) as cpool,
            tc.tile_pool(name="batch", bufs=2) as bpool,
            tc.tile_pool(name="x", bufs=8) as xpool,
            tc.tile_pool(name="y", bufs=6) as ypool,
            tc.tile_pool(name="o", bufs=3) as opool,
            tc.tile_pool(name="pz", bufs=2, space="PSUM") as pzpool,
            tc.tile_pool(name="po", bufs=2, space="PSUM") as popool,
        ):
            w2t = cpool.tile([D, 128], F16)
            nc.sync.dma_start(w2t[:], w2d.ap()[:, :])
            w3t = cpool.tile([D, 4], F16)
            nc.sync.dma_start(w3t[:], w3d.ap()[:, :])
            b1t = cpool.tile([D, 1], F32)
            nc.sync.dma_start(b1t[:], b1r.ap()[:, :])
            b2t = cpool.tile([D, 1], F32)
            nc.sync.dma_start(b2t[:], b2r.ap()[:, :])

            for b in range(BPC):
                hh = bpool.tile([D, NP], F16, tag="hh")
                nc.sync.dma_start(hh[:], hhT.ap()[b, :, :])
                hn = bpool.tile([D, NP], F16, tag="hn")
                nc.sync.dma_start(hn[:], hnT.ap()[b, :, :])
                g1at = bpool.tile([D, 32], F16, tag="g1a")
                nc.sync.dma_start(g1at[:], g1a.ap()[b, :, :])
                g2at = bpool.tile([D, 32], F16, tag="g2a")
                nc.sync.dma_start(g2at[:], g2a.ap()[b, :, :])
                g1ct = bpool.tile([D, 128], F16, tag="g1c")
                nc.sync.dma_start(g1ct[:], g1c.ap()[b, :, :])
                g2ct = bpool.tile([D, 128], F16, tag="g2c")
                nc.sync.dma_start(g2ct[:], g2c.ap()[b, :, :])

                # C^T replicated 4x on partitions (128 x NP) and A^T (32 x NP),
                # in PSUM tiles riding the pz ring.
                pac = pzpool.tile([D, 2 * NP], F32, tag="pz")
                nc.tensor.matmul(pac[:, 0:NP], g1ct[:], hh[:], start=True, stop=False)
                nc.tensor.matmul(pac[:, 0:NP], g2ct[:], hn[:], start=False, stop=True)
                paa = pzpool.tile([D, 2 * NP], F32, tag="pz")
                nc.tensor.matmul(
                    paa[0:32, 0:NP], g1at[:], hh[:], start=True, stop=False
                )
                nc.tensor.matmul(
                    paa[0:32, 0:NP], g2at[:], hn[:], start=False, stop=True
                )

                # crep = C^T(rep4) + b1   (bias folded in)
                crep = bpool.tile([D, NP], F16, tag="crep")
                nc.vector.tensor_scalar_add(crep[:], pac[:, 0:NP], b1t[:, 0:1])

                # a4[32r+k, t] = A^T[k, 4t+r]  (i-tile column layout)
                a4 = bpool.tile([D, NT], F32, tag="a4")
                paa_r = paa[0:32, 0 : 4 * NT].rearrange("p (t r) -> p r t", r=4)
                for r in range(4):
                    nc.scalar.copy(a4[32 * r : 32 * r + 32, :], paa_r[:, r, :])

                # main pair loop, software-pipelined with per-stage skews so
                # no engine's in-order queue ever waits on another engine's
                # freshest output:
                #   iter i:  X,X for pair i (DVE)
                #            fc2 matmuls for pair i-1 (PE)
                #            Y2 wide op for pair i-2 (ACT / DVE)
                #            fc3 matmuls for pair i-3 (PE) + copy/DMA per group
                pend_x = {}
                pend_pz = {}
                pend_y = {}
                po2 = None
                for p in range(PAIRS + 3):
                    if p < PAIRS:
                        t0 = 2 * p
                        xs = []
                        for t in (t0, t0 + 1):
                            x = xpool.tile([D, NP], F16, tag="x")
                            nc.vector.tensor_scalar(
                                out=x[:],
                                in0=crep[:],
                                scalar1=a4[:, t : t + 1],
                                scalar2=0.0,
                                op0=add,
                                op1=amax,
                            )
                            xs.append(x)
                        pend_x[p] = xs

                    pm = p - 1
                    if 0 <= pm < PAIRS:
                        xs = pend_x.pop(pm)
                        pz2 = pzpool.tile([D, 2 * NP], F32, tag="pz")
                        nc.tensor.matmul(
                            pz2[:, 0:NP], w2t[:], xs[0][:], start=True, stop=True
                        )
                        nc.tensor.matmul(
                            pz2[:, NP : 2 * NP], w2t[:], xs[1][:], start=True, stop=True
                        )
                        pend_pz[pm] = pz2

                    py = p - 2
                    if 0 <= py < PAIRS:
                        pz2 = pend_pz.pop(py)
                        y2 = ypool.tile([D, 2 * NP], F16, tag="y2")
                        if py % Y2_DVE_EVERY == Y2_DVE_EVERY - 1:
                            nc.vector.tensor_scalar(
                                out=y2[:],
                                in0=pz2[:],
                                scalar1=b2t[:, 0:1],
                                scalar2=0.0,
                                op0=add,
                                op1=amax,
                            )
                        else:
                            nc.scalar.activation(y2[:], pz2[:], Relu, bias=b2t[:, 0:1])
                        pend_y[py] = y2

                    pf = p - 3
                    if 0 <= pf < PAIRS:
                        s, q = divmod(pf, 4)
                        if q == 0:
                            po2 = popool.tile([D, 2 * NP], F32, tag="po")
                        y2 = pend_y.pop(pf)
                        for j in (0, 1):
                            st = 2 * q + j
                            h, v = st // 4, st % 4
                            nc.tensor.matmul(
                                po2[32 * v : 32 * v + 4, h * NP : h * NP + NP],
                                w3t[:],
                                y2[:, j * NP : j * NP + NP],
                                start=True,
                                stop=True,
                                tile_position=(0, 32 * v),
                            )
                        if q == 3 or pf == PAIRS - 1:
                            # used fc3 partitions are {32v+r, r<4} (v = slot%4,
                            # max 99); dump them raw, host un-permutes rows.
                            ob2 = opool.tile([D, 2 * NP], F32, tag="ob")
                            nc.scalar.copy(ob2[0:100, :], po2[0:100, :])
                            nc.sync.dma_start(raw.ap()[b, s, :, :], ob2[0:100, :])

    nc.compile()
    return nc


def _host_prep(h_hat, pos_pickup, pos_delivery, solution, Wq1, Wk1, Wq2, Wk2, fc1_w):
    """Per-batch tiny maps G (128x32 each) + transposed/padded node features."""
    f32 = np.float32
    h_hat = np.asarray(h_hat, f32)
    pp = np.asarray(pos_pickup).astype(np.int64)
    pd = np.asarray(pos_delivery).astype(np.int64)
    sol = np.asarray(solution).astype(np.int64)
    Wq1 = np.asarray(Wq1, f32)
    Wk1 = np.asarray(Wk1, f32)
    Wq2 = np.asarray(Wq2, f32)
    Wk2 = np.asarray(Wk2, f32)
    fc1_w = np.asarray(fc1_w, f32)

    hhT = np.zeros((B, D, NP), f32)
    hnT = np.zeros((B, D, NP), f32)
    g1a = np.zeros((B, D, 32), f32)
    g2a = np.zeros((B, D, 32), f32)
    g1c = np.zeros((B, D, 128), f32)
    g2c = np.zeros((B, D, 128), f32)

    for b in range(B):
        hb = h_hat[b]  # (N, D)
        hnb = hb[sol[b]]  # (N, D) gathered neighbours
        hhT[b, :, :N] = hb.T
        hnT[b, :, :N] = hnb.T
        p = hb[pp[b]]  # (D,)
        dv = hb[pd[b]]
        # u[h] = Wk[h] @ (Wq[h]^T @ q): compat[n,h] = x[n] . u[h]
        U1p = np.stack([Wk1[h] @ (Wq1[h].T @ p) for h in range(H)], axis=1)
        U2p = np.stack([Wk2[h] @ (Wq2[h].T @ p) for h in range(H)], axis=1)
        U1d = np.stack([Wk1[h] @ (Wq1[h].T @ dv) for h in range(H)], axis=1)
        U2d = np.stack([Wk2[h] @ (Wq2[h].T @ dv) for h in range(H)], axis=1)
        g1a[b] = U1p @ fc1_w[0:4]  # h_hat -> A
        g2a[b] = U2p @ fc1_w[4:8]  # h_nb  -> A
        gc1 = U1d @ fc1_w[8:12]  # h_hat -> C
        gc2 = U2d @ fc1_w[12:16]  # h_nb  -> C
        g1c[b] = np.tile(gc1, (1, 4))
        g2c[b] = np.tile(gc2, (1, 4))
    return hhT, hnT, g1a, g2a, g1c, g2c


_last_results = None


def kernel(
    h_hat,
    pos_pickup,
    pos_delivery,
    solution,
    Wq1,
    Wk1,
    Wq2,
    Wk2,
    fc1_w,
    fc1_b,
    fc2_w,
    fc2_b,
    fc3_w,
    fc3_b,
):
    global _last_results
    from concourse.bass_utils import run_bass_kernel_spmd

    f32 = np.float32
    fc2_w = np.asarray(fc2_w, f32)
    fc1_b = np.asarray(fc1_b, f32)
    fc2_b = np.asarray(fc2_b, f32)
    fc3_w = np.asarray(fc3_w, f32)
    fc3_b = np.asarray(fc3_b, f32)

    hhT, hnT, g1a, g2a, g1c, g2c = _host_prep(
        h_hat, pos_pickup, pos_delivery, solution, Wq1, Wk1, Wq2, Wk2,
        np.asarray(fc1_w, f32),
    )

    # block-diagonal packed MLP weights (4 independent 32-blocks)
    w2d = np.zeros((D, 128), f32)
    w3d = np.zeros((D, 4), f32)
    for r in range(4):
        w2d[32 * r : 32 * r + 32, 32 * r : 32 * r + 32] = fc2_w
        w3d[32 * r : 32 * r + 32, r : r + 1] = fc3_w.reshape(32, 1)
    b1r = np.tile(fc1_b.reshape(32, 1), (4, 1)).astype(f32)
    b2r = np.tile(fc2_b.reshape(32, 1), (4, 1)).astype(f32)

    if "nc" not in _cache:
        _cache["nc"] = _build_program()
    nc = _cache["nc"]

    in_maps = []
    for c in range(NCORES):
        bs = slice(BPC * c, BPC * (c + 1))
        in_maps.append(
            {
                "hhT": np.ascontiguousarray(hhT[bs]).astype(np.float16),
                "hnT": np.ascontiguousarray(hnT[bs]).astype(np.float16),
                "g1a": np.ascontiguousarray(g1a[bs]).astype(np.float16),
                "g2a": np.ascontiguousarray(g2a[bs]).astype(np.float16),
                "g1c": np.ascontiguousarray(g1c[bs]).astype(np.float16),
                "g2c": np.ascontiguousarray(g2c[bs]).astype(np.float16),
                "w2d": w2d.astype(np.float16),
                "w3d": w3d.astype(np.float16),
                "b1r": b1r,
                "b2r": b2r,
            }
        )

    res = run_bass_kernel_spmd(nc, in_maps, core_ids=list(range(NCORES)))
    _last_results = res

    # un-permute: raw[b, s, 32v+r, 512h+j] holds out row 32s+16h+4v+r
    rows = np.arange(N)
    s_i = rows // 32
    rem = rows % 32
    h_i = rem // 16
    rem2 = rem % 16
    part = 32 * (rem2 // 4) + (rem2 % 4)
    foff = NP * h_i
    cols = np.arange(N)
    out = np.empty((B, N, N), f32)
    for c in range(NCORES):
        rawc = res.results[c]["raw"]  # [BPC, NSG, 100, 2*NP]
        for bb in range(BPC):
            out[BPC * c + bb] = rawc[bb, s_i[:, None], part[:, None], foff[:, None] + cols[None, :]]
    b3 = float(fc3_b.reshape(-1)[0])
    if b3 != 0.0:
        out = out + b3
    return out.astype(f32)


# revision 7
# speedup vs baseline: 3.9459x; 1.1626x over previous
"""Trainium2 Bass kernel for nn_NNSDecoder (gnn_message_passing).

Reference computation (B=16, N=501, D=128, H=4):
    out[b,i,j] = fc3 . relu(fc2^T relu(feat @ fc1 + b1) + b2) + b3
    feat[b,i,j] = [cp_pre[b,i], cp_post[b,i], cd_pre[b,j], cd_post[b,j]]  (4H=16)

Key algebra: compat[b,n,h] = x[b,n] . (Wk[h] Wq[h]^T q_b), so every
pickup/delivery-side term is linear in h_hat / h_nb rows.  Folding the
head projections and fc1 together gives per-batch 128x32 maps:
    A[b] = h_hat[b] @ G_A1 + h_nb[b] @ G_A2          (N x 32, row/i term)
    C[b] = h_hat[b] @ G_C1 + h_nb[b] @ G_C2          (N x 32, col/j term)
    out[b,i,j] = w3 . relu(W2^T relu(A[b,i] + C[b,j] + b1) + b2) + b3

The tiny G matrices (and the index gathers defining them) are computed on
host; the device does all O(N) and O(N^2) work.  Per 4-row i-tile t:
    X_t = relu(crep + a4[:,t])            (DVE tensor_scalar, f16, 2x mode)
    Z_t = W2blk @ X_t                     (PE matmul, block-diag f16)
    Y_t = relu(Z_t + b2)                  (ScalarE ACT / DVE, f16)
    po  = w3blk @ Y_t                     (PE matmul into packed PSUM)
i-tiles are processed in PAIRS sharing one 2-bank PSUM tile [128,1024]
(NP=512 = exact bank) so Y and the PSUM->SBUF copy run as single wide
ops; fc3 matmuls are emitted 2 pairs late (software pipelining) so the
PE never stalls waiting for Y; 8-tile supergroups drain with one wide
copy + 2 batched 16-row DMAs (partition-strided APs).

Sharding: batch dim 16 -> 8 cores x 2 batches (data parallel, weights
replicated). Full inputs in, full output out.
"""

import numpy as np

B, N, D, H = 16, 501, 128, 4
NCORES = 8
BPC = B // NCORES  # batches per core
NP = 512  # padded j: exact PSUM bank (512 f32 = 2KB)
NT = 126  # i-tiles of 4 rows (126*4 = 504 >= 501)
PAIRS = NT // 2  # 63 i-tile pairs per batch

# engine split knob: every Y2_DVE_EVERY-th pair's Y runs on DVE, rest ScalarE
Y2_DVE_EVERY = 1000000  # all Y2 on ScalarE: DVE-Y2 blocks X production (in-order queue)

_cache = {}


def _build_program():
    import concourse.bacc as bacc
    import concourse.mybir as mybir
    from concourse.tile import TileContext

    F32 = mybir.dt.float32
    F32R = mybir.dt.float32r
    F16 = mybir.dt.float16
    nc = bacc.Bacc("TRN2", target_bir_lowering=False, debug=False, num_devices=1)

    hhT = nc.dram_tensor("hhT", [BPC, D, NP], F16, kind="ExternalInput")
    hnT = nc.dram_tensor("hnT", [BPC, D, NP], F16, kind="ExternalInput")
    g1a = nc.dram_tensor("g1a", [BPC, D, 32], F16, kind="ExternalInput")
    g2a = nc.dram_tensor("g2a", [BPC, D, 32], F16, kind="ExternalInput")
    g1c = nc.dram_tensor("g1c", [BPC, D, 128], F16, kind="ExternalInput")
    g2c = nc.dram_tensor("g2c", [BPC, D, 128], F16, kind="ExternalInput")
    w2d = nc.dram_tensor("w2d", [D, 128], F16, kind="ExternalInput")
    w3d = nc.dram_tensor("w3d", [D, 4], F16, kind="ExternalInput")
    b1r = nc.dram_tensor("b1r", [D, 1], F32, kind="ExternalInput")
    b2r = nc.dram_tensor("b2r", [D, 1], F32, kind="ExternalInput")
    NSG = 16  # supergroups per batch (8 i-tiles / 32 rows each)
    raw = nc.dram_tensor("raw", [BPC, NSG, 100, 2 * NP], F32, kind="ExternalOutput")

    add = mybir.AluOpType.add
    amax = mybir.AluOpType.max
    Relu = mybir.ActivationFunctionType.Relu

    with TileContext(nc) as tc:
        with (
            tc.tile_pool(name="const", bufs=1) as cpool,
            tc.tile_pool(name="batch", bufs=2) as bpool,
            tc.tile_pool(name="x", bufs=8) as xpool,
            tc.tile_pool(name="y", bufs=6) as ypool,
            tc.tile_pool(name="o", bufs=3) as opool,
            tc.tile_pool(name="pz", bufs=2, space="PSUM") as pzpool,
            tc.tile_pool(name="po", bufs=2, space="PSUM") as popool,
        ):
            w2t = cpool.tile([D, 128], F16)
            nc.sync.dma_start(w2t[:], w2d.ap()[:, :])
            w3t = cpool.tile([D, 4], F16)
            nc.sync.dma_start(w3t[:], w3d.ap()[:, :])
            b1t = cpool.tile([D, 1], F32)
            nc.sync.dma_start(b1t[:], b1r.ap()[:, :])
            b2t = cpool.tile([D, 1], F32)
            nc.sync.dma_start(b2t[:], b2r.ap()[:, :])

            for b in range(BPC):
                hh = bpool.tile([D, NP], F16, tag="hh")
                nc.sync.dma_start(hh[:], hhT.ap()[b, :, :])
                hn = bpool.tile([D, NP], F16, tag="hn")
                nc.sync.dma_start(hn[:], hnT.ap()[b, :, :])
                g1at = bpool.tile([D, 32], F16, tag="g1a")
                nc.sync.dma_start(g1at[:], g1a.ap()[b, :, :])
                g2at = bpool.tile([D, 32], F16, tag="g2a")
                nc.sync.dma_start(g2at[:], g2a.ap()[b, :, :])
                g1ct = bpool.tile([D, 128], F16, tag="g1c")
                nc.sync.dma_start(g1ct[:], g1c.ap()[b, :, :])
                g2ct = bpool.tile([D, 128], F16, tag="g2c")
                nc.sync.dma_start(g2ct[:], g2c.ap()[b, :, :])

                # C^T replicated 4x on partitions (128 x NP) and A^T (32 x NP),
                # in PSUM tiles riding the pz ring.
                pac = pzpool.tile([D, 2 * NP], F32, tag="pz")
                nc.tensor.matmul(pac[:, 0:NP], g1ct[:], hh[:], start=True, stop=False)
                nc.tensor.matmul(pac[:, 0:NP], g2ct[:], hn[:], start=False, stop=True)
                paa = pzpool.tile([D, 2 * NP], F32, tag="pz")
                nc.tensor.matmul(
                    paa[0:32, 0:NP], g1at[:], hh[:], start=True, stop=False
                )
                nc.tensor.matmul(
                    paa[0:32, 0:NP], g2at[:], hn[:], start=False, stop=True
                )

                # crep = C^T(rep4) + b1   (bias folded in)
                crep = bpool.tile([D, NP], F16, tag="crep")
                nc.vector.tensor_scalar_add(crep[:], pac[:, 0:NP], b1t[:, 0:1])

                # a4[32r+k, t] = A^T[k, 4t+r]  (i-tile column layout)
                a4 = bpool.tile([D, NT], F32, tag="a4")
                paa_r = paa[0:32, 0 : 4 * NT].rearrange("p (t r) -> p r t", r=4)
                for r in range(4):
                    nc.scalar.copy(a4[32 * r : 32 * r + 32, :], paa_r[:, r, :])

                # main pair loop, software-pipelined with per-stage skews so
                # no engine's in-order queue ever waits on another engine's
                # freshest output:
                #   iter i:  X,X for pair i (DVE)
                #            fc2 matmuls for pair i-1 (PE)
                #            Y2 wide op for pair i-2 (ACT / DVE)
                #            fc3 matmuls for pair i-3 (PE) + copy/DMA per group
                pend_x = {}
                pend_pz = {}
                pend_y = {}
                po2 = None
                for p in range(PAIRS + 3):
                    if p < PAIRS:
                        t0 = 2 * p
                        xs = []
                        for t in (t0, t0 + 1):
                            x = xpool.tile([D, NP], F16, tag="x")
                            nc.vector.tensor_scalar(
                                out=x[:],
                                in0=crep[:],
                                scalar1=a4[:, t : t + 1],
                                scalar2=0.0,
                                op0=add,
                                op1=amax,
                            )
                            xs.append(x)
                        pend_x[p] = xs

                    pm = p - 1
                    if 0 <= pm < PAIRS:
                        xs = pend_x.pop(pm)
                        pz2 = pzpool.tile([D, 2 * NP], F32, tag="pz")
                        nc.tensor.matmul(
                            pz2[:, 0:NP], w2t[:], xs[0][:], start=True, stop=True
                        )
                        nc.tensor.matmul(
                            pz2[:, NP : 2 * NP], w2t[:], xs[1][:], start=True, stop=True
                        )
                        pend_pz[pm] = pz2

                    py = p - 2
                    if 0 <= py < PAIRS:
                        pz2 = pend_pz.pop(py)
                        y2 = ypool.tile([D, 2 * NP], F16, tag="y2")
                        if py % Y2_DVE_EVERY == Y2_DVE_EVERY - 1:
                            nc.vector.tensor_scalar(
                                out=y2[:],
                                in0=pz2[:],
                                scalar1=b2t[:, 0:1],
                                scalar2=0.0,
                                op0=add,
                                op1=amax,
                            )
                        else:
                            nc.scalar.activation(y2[:], pz2[:], Relu, bias=b2t[:, 0:1])
                        pend_y[py] = y2

                    pf = p - 3
                    if 0 <= pf < PAIRS:
                        s, q = divmod(pf, 4)
                        if q == 0:
                            po2 = popool.tile([D, 2 * NP], F32, tag="po")
                        y2 = pend_y.pop(pf)
                        for j in (0, 1):
                            st = 2 * q + j
                            h, v = st // 4, st % 4
                            nc.tensor.matmul(
                                po2[32 * v : 32 * v + 4, h * NP : h * NP + NP],
                                w3t[:],
                                y2[:, j * NP : j * NP + NP],
                                start=True,
                                stop=True,
                                tile_position=(0, 32 * v),
                            )
                        if q == 3 or pf == PAIRS - 1:
                            # used fc3 partitions are {32v+r, r<4} (v = slot%4,
                            # max 99); dump them raw, host un-permutes rows.
                            ob2 = opool.tile([D, 2 * NP], F32, tag="ob")
                            nc.vector.tensor_scalar_add(ob2[0:100, :], po2[0:100, :], 0.0)
                            nc.sync.dma_start(raw.ap()[b, s, :, :], ob2[0:100, :])

    nc.compile()
    return nc


def _host_prep(h_hat, pos_pickup, pos_delivery, solution, Wq1, Wk1, Wq2, Wk2, fc1_w):
    """Per-batch tiny maps G (128x32 each) + transposed/padded node features."""
    f32 = np.float32
    h_hat = np.asarray(h_hat, f32)
    pp = np.asarray(pos_pickup).astype(np.int64)
    pd = np.asarray(pos_delivery).astype(np.int64)
    sol = np.asarray(solution).astype(np.int64)
    Wq1 = np.asarray(Wq1, f32)
    Wk1 = np.asarray(Wk1, f32)
    Wq2 = np.asarray(Wq2, f32)
    Wk2 = np.asarray(Wk2, f32)
    fc1_w = np.asarray(fc1_w, f32)

    hhT = np.zeros((B, D, NP), f32)
    hnT = np.zeros((B, D, NP), f32)
    g1a = np.zeros((B, D, 32), f32)
    g2a = np.zeros((B, D, 32), f32)
    g1c = np.zeros((B, D, 128), f32)
    g2c = np.zeros((B, D, 128), f32)

    for b in range(B):
        hb = h_hat[b]  # (N, D)
        hnb = hb[sol[b]]  # (N, D) gathered neighbours
        hhT[b, :, :N] = hb.T
        hnT[b, :, :N] = hnb.T
        p = hb[pp[b]]  # (D,)
        dv = hb[pd[b]]
        # u[h] = Wk[h] @ (Wq[h]^T @ q): compat[n,h] = x[n] . u[h]
        U1p = np.stack([Wk1[h] @ (Wq1[h].T @ p) for h in range(H)], axis=1)
        U2p = np.stack([Wk2[h] @ (Wq2[h].T @ p) for h in range(H)], axis=1)
        U1d = np.stack([Wk1[h] @ (Wq1[h].T @ dv) for h in range(H)], axis=1)
        U2d = np.stack([Wk2[h] @ (Wq2[h].T @ dv) for h in range(H)], axis=1)
        g1a[b] = U1p @ fc1_w[0:4]  # h_hat -> A
        g2a[b] = U2p @ fc1_w[4:8]  # h_nb  -> A
        gc1 = U1d @ fc1_w[8:12]  # h_hat -> C
        gc2 = U2d @ fc1_w[12:16]  # h_nb  -> C
        g1c[b] = np.tile(gc1, (1, 4))
        g2c[b] = np.tile(gc2, (1, 4))
    return hhT, hnT, g1a, g2a, g1c, g2c


_last_results = None


def kernel(
    h_hat,
    pos_pickup,
    pos_delivery,
    solution,
    Wq1,
    Wk1,
    Wq2,
    Wk2,
    fc1_w,
    fc1_b,
    fc2_w,
    fc2_b,
    fc3_w,
    fc3_b,
):
    global _last_results
    from concourse.bass_utils import run_bass_kernel_spmd

    f32 = np.float32
    fc2_w = np.asarray(fc2_w, f32)
    fc1_b = np.asarray(fc1_b, f32)
    fc2_b = np.asarray(fc2_b, f32)
    fc3_w = np.asarray(fc3_w, f32)
    fc3_b = np.asarray(fc3_b, f32)

    hhT, hnT, g1a, g2a, g1c, g2c = _host_prep(
        h_hat, pos_pickup, pos_delivery, solution, Wq1, Wk1, Wq2, Wk2,
        np.asarray(fc1_w, f32),
    )

    # block-diagonal packed MLP weights (4 independent 32-blocks)
    w2d = np.zeros((D, 128), f32)
    w3d = np.zeros((D, 4), f32)
    for r in range(4):
        w2d[32 * r : 32 * r + 32, 32 * r : 32 * r + 32] = fc2_w
        w3d[32 * r : 32 * r + 32, r : r + 1] = fc3_w.reshape(32, 1)
    b1r = np.tile(fc1_b.reshape(32, 1), (4, 1)).astype(f32)
    b2r = np.tile(fc2_b.reshape(32, 1), (4, 1)).astype(f32)

    if "nc" not in _cache:
        _cache["nc"] = _build_program()
    nc = _cache["nc"]

    in_maps = []
    for c in range(NCORES):
        bs = slice(BPC * c, BPC * (c + 1))
        in_maps.append(
            {
                "hhT": np.ascontiguousarray(hhT[bs]).astype(np.float16),
                "hnT": np.ascontiguousarray(hnT[bs]).astype(np.float16),
                "g1a": np.ascontiguousarray(g1a[bs]).astype(np.float16),
                "g2a": np.ascontiguousarray(g2a[bs]).astype(np.float16),
                "g1c": np.ascontiguousarray(g1c[bs]).astype(np.float16),
                "g2c": np.ascontiguousarray(g2c[bs]).astype(np.float16),
                "w2d": w2d.astype(np.float16),
                "w3d": w3d.astype(np.float16),
                "b1r": b1r,
                "b2r": b2r,
            }
        )

    res = run_bass_kernel_spmd(nc, in_maps, core_ids=list(range(NCORES)))
    _last_results = res

    # un-permute: raw[b, s, 32v+r, 512h+j] holds out row 32s+16h+4v+r
    rows = np.arange(N)
    s_i = rows // 32
    rem = rows % 32
    h_i = rem // 16
    rem2 = rem % 16
    part = 32 * (rem2 // 4) + (rem2 % 4)
    foff = NP * h_i
    cols = np.arange(N)
    out = np.empty((B, N, N), f32)
    for c in range(NCORES):
        rawc = res.results[c]["raw"]  # [BPC, NSG, 100, 2*NP]
        for bb in range(BPC):
            out[BPC * c + bb] = rawc[bb, s_i[:, None], part[:, None], foff[:, None] + cols[None, :]]
    b3 = float(fc3_b.reshape(-1)[0])
    if b3 != 0.0:
        out = out + b3
    return out.astype(f32)


# revision 8
# speedup vs baseline: 3.9553x; 1.0024x over previous
"""Trainium2 Bass kernel for nn_NNSDecoder (gnn_message_passing).

Reference computation (B=16, N=501, D=128, H=4):
    out[b,i,j] = fc3 . relu(fc2^T relu(feat @ fc1 + b1) + b2) + b3
    feat[b,i,j] = [cp_pre[b,i], cp_post[b,i], cd_pre[b,j], cd_post[b,j]]  (4H=16)

Key algebra: compat[b,n,h] = x[b,n] . (Wk[h] Wq[h]^T q_b), so every
pickup/delivery-side term is linear in h_hat / h_nb rows.  Folding the
head projections and fc1 together gives per-batch 128x32 maps:
    A[b] = h_hat[b] @ G_A1 + h_nb[b] @ G_A2          (N x 32, row/i term)
    C[b] = h_hat[b] @ G_C1 + h_nb[b] @ G_C2          (N x 32, col/j term)
    out[b,i,j] = w3 . relu(W2^T relu(A[b,i] + C[b,j] + b1) + b2) + b3

The tiny G matrices (and the index gathers defining them) are computed on
host; the device does all O(N) and O(N^2) work.  Per 4-row i-tile t:
    X_t = relu(crep + a4[:,t])            (DVE tensor_scalar, f16, 2x mode)
    Z_t = W2blk @ X_t                     (PE matmul, block-diag f16)
    Y_t = relu(Z_t + b2)                  (ScalarE ACT / DVE, f16)
    po  = w3blk @ Y_t                     (PE matmul into packed PSUM)
i-tiles are processed in PAIRS sharing one 2-bank PSUM tile [128,1024]
(NP=512 = exact bank) so Y and the PSUM->SBUF copy run as single wide
ops; fc3 matmuls are emitted 2 pairs late (software pipelining) so the
PE never stalls waiting for Y; 8-tile supergroups drain with one wide
copy + 2 batched 16-row DMAs (partition-strided APs).

Sharding: batch dim 16 -> 8 cores x 2 batches (data parallel, weights
replicated). Full inputs in, full output out.
"""

import numpy as np

B, N, D, H = 16, 501, 128, 4
NCORES = 8
BPC = B // NCORES  # batches per core
NP = 512  # padded j: exact PSUM bank (512 f32 = 2KB)
NT = 126  # i-tiles of 4 rows (126*4 = 504 >= 501)
PAIRS = NT // 2  # 63 i-tile pairs per batch

# engine split knob: every Y2_DVE_EVERY-th pair's Y runs on DVE, rest ScalarE
Y2_DVE_EVERY = 1000000  # all Y2 on ScalarE: DVE-Y2 blocks X production (in-order queue)

_cache = {}


def _build_program():
    import concourse.bacc as bacc
    import concourse.mybir as mybir
    from concourse.tile import TileContext

    F32 = mybir.dt.float32
    F32R = mybir.dt.float32r
    F16 = mybir.dt.float16
    nc = bacc.Bacc("TRN2", target_bir_lowering=False, debug=False, num_devices=1)

    hhT = nc.dram_tensor("hhT", [BPC, D, NP], F16, kind="ExternalInput")
    hnT = nc.dram_tensor("hnT", [BPC, D, NP], F16, kind="ExternalInput")
    g1a = nc.dram_tensor("g1a", [BPC, D, 32], F16, kind="ExternalInput")
    g2a = nc.dram_tensor("g2a", [BPC, D, 32], F16, kind="ExternalInput")
    g1c = nc.dram_tensor("g1c", [BPC, D, 128], F16, kind="ExternalInput")
    g2c = nc.dram_tensor("g2c", [BPC, D, 128], F16, kind="ExternalInput")
    w2d = nc.dram_tensor("w2d", [D, 128], F16, kind="ExternalInput")
    w3d = nc.dram_tensor("w3d", [D, 4], F16, kind="ExternalInput")
    b1r = nc.dram_tensor("b1r", [D, 1], F32, kind="ExternalInput")
    b2r = nc.dram_tensor("b2r", [D, 1], F32, kind="ExternalInput")
    NSG = 16  # supergroups per batch (8 i-tiles / 32 rows each)
    raw = nc.dram_tensor("raw", [BPC, NSG, 100, 2 * NP], F32, kind="ExternalOutput")

    add = mybir.AluOpType.add
    amax = mybir.AluOpType.max
    Relu = mybir.ActivationFunctionType.Relu

    with TileContext(nc) as tc:
        with (
            tc.tile_pool(name="const", bufs=1) as cpool,
            tc.tile_pool(name="batch", bufs=2) as bpool,
            tc.tile_pool(name="x", bufs=8) as xpool,
            tc.tile_pool(name="y", bufs=6) as ypool,
            tc.tile_pool(name="o", bufs=3) as opool,
            tc.tile_pool(name="pz", bufs=2, space="PSUM") as pzpool,
            tc.tile_pool(name="po", bufs=2, space="PSUM") as popool,
        ):
            w2t = cpool.tile([D, 128], F16)
            nc.sync.dma_start(w2t[:], w2d.ap()[:, :])
            w3t = cpool.tile([D, 4], F16)
            nc.sync.dma_start(w3t[:], w3d.ap()[:, :])
            b1t = cpool.tile([D, 1], F32)
            nc.sync.dma_start(b1t[:], b1r.ap()[:, :])
            b2t = cpool.tile([D, 1], F32)
            nc.sync.dma_start(b2t[:], b2r.ap()[:, :])

            for b in range(BPC):
                hh = bpool.tile([D, NP], F16, tag="hh")
                nc.sync.dma_start(hh[:], hhT.ap()[b, :, :])
                hn = bpool.tile([D, NP], F16, tag="hn")
                nc.sync.dma_start(hn[:], hnT.ap()[b, :, :])
                g1at = bpool.tile([D, 32], F16, tag="g1a")
                nc.sync.dma_start(g1at[:], g1a.ap()[b, :, :])
                g2at = bpool.tile([D, 32], F16, tag="g2a")
                nc.sync.dma_start(g2at[:], g2a.ap()[b, :, :])
                g1ct = bpool.tile([D, 128], F16, tag="g1c")
                nc.sync.dma_start(g1ct[:], g1c.ap()[b, :, :])
                g2ct = bpool.tile([D, 128], F16, tag="g2c")
                nc.sync.dma_start(g2ct[:], g2c.ap()[b, :, :])

                # C^T replicated 4x on partitions (128 x NP) and A^T (32 x NP),
                # in PSUM tiles riding the pz ring.
                pac = pzpool.tile([D, 2 * NP], F32, tag="pz")
                nc.tensor.matmul(pac[:, 0:NP], g1ct[:], hh[:], start=True, stop=False)
                nc.tensor.matmul(pac[:, 0:NP], g2ct[:], hn[:], start=False, stop=True)
                paa = pzpool.tile([D, 2 * NP], F32, tag="pz")
                nc.tensor.matmul(
                    paa[0:32, 0:NP], g1at[:], hh[:], start=True, stop=False
                )
                nc.tensor.matmul(
                    paa[0:32, 0:NP], g2at[:], hn[:], start=False, stop=True
                )

                # crep = C^T(rep4) + b1   (bias folded in)
                crep = bpool.tile([D, NP], F16, tag="crep")
                nc.vector.tensor_scalar_add(crep[:], pac[:, 0:NP], b1t[:, 0:1])

                # a4[32r+k, t] = A^T[k, 4t+r]  (i-tile column layout)
                a4 = bpool.tile([D, NT], F32, tag="a4")
                paa_r = paa[0:32, 0 : 4 * NT].rearrange("p (t r) -> p r t", r=4)
                for r in range(4):
                    nc.scalar.copy(a4[32 * r : 32 * r + 32, :], paa_r[:, r, :])

                # main pair loop, software-pipelined with per-stage skews so
                # no engine's in-order queue ever waits on another engine's
                # freshest output:
                #   iter i:  X,X for pair i (DVE)
                #            fc2 matmuls for pair i-1 (PE)
                #            Y2 wide op for pair i-2 (ACT / DVE)
                #            fc3 matmuls for pair i-3 (PE) + copy/DMA per group
                pend_x = {}
                pend_pz = {}
                pend_y = {}
                pend_po = {}
                po2 = None
                for p in range(PAIRS + 3 + 5):
                    if p < PAIRS:
                        t0 = 2 * p
                        xs = []
                        for t in (t0, t0 + 1):
                            x = xpool.tile([D, NP], F16, tag="x")
                            nc.vector.tensor_scalar(
                                out=x[:],
                                in0=crep[:],
                                scalar1=a4[:, t : t + 1],
                                scalar2=0.0,
                                op0=add,
                                op1=amax,
                            )
                            xs.append(x)
                        pend_x[p] = xs

                    pm = p - 1
                    if 0 <= pm < PAIRS:
                        xs = pend_x.pop(pm)
                        pz2 = pzpool.tile([D, 2 * NP], F32, tag="pz")
                        nc.tensor.matmul(
                            pz2[:, 0:NP], w2t[:], xs[0][:], start=True, stop=True
                        )
                        nc.tensor.matmul(
                            pz2[:, NP : 2 * NP], w2t[:], xs[1][:], start=True, stop=True
                        )
                        pend_pz[pm] = pz2

                    py = p - 2
                    if 0 <= py < PAIRS:
                        pz2 = pend_pz.pop(py)
                        y2 = ypool.tile([D, 2 * NP], F16, tag="y2")
                        if py % Y2_DVE_EVERY == Y2_DVE_EVERY - 1:
                            nc.vector.tensor_scalar(
                                out=y2[:],
                                in0=pz2[:],
                                scalar1=b2t[:, 0:1],
                                scalar2=0.0,
                                op0=add,
                                op1=amax,
                            )
                        else:
                            nc.scalar.activation(y2[:], pz2[:], Relu, bias=b2t[:, 0:1])
                        pend_y[py] = y2

                    pf = p - 3
                    if 0 <= pf < PAIRS:
                        s, q = divmod(pf, 4)
                        if q == 0:
                            po2 = popool.tile([D, 2 * NP], F32, tag="po")
                        y2 = pend_y.pop(pf)
                        for j in (0, 1):
                            st = 2 * q + j
                            h, v = st // 4, st % 4
                            nc.tensor.matmul(
                                po2[32 * v : 32 * v + 4, h * NP : h * NP + NP],
                                w3t[:],
                                y2[:, j * NP : j * NP + NP],
                                start=True,
                                stop=True,
                                tile_position=(0, 32 * v),
                            )
                        if q == 3 or pf == PAIRS - 1:
                            pend_po[s] = po2

                    # copy/DMA two iters after a supergroup's last fc3, so the
                    # DVE copy never waits mid-queue and never delays X ops
                    pc = p - 8
                    if pc >= 0 and pc % 4 == 0 and (pc // 4) in pend_po:
                        s2 = pc // 4
                        po2c = pend_po.pop(s2)
                        # used fc3 partitions are {32v+r, r<4} (v = slot%4,
                        # max 99); dump them raw, host un-permutes rows.
                        ob2 = opool.tile([D, 2 * NP], F32, tag="ob")
                        nc.vector.tensor_scalar_add(ob2[0:100, :], po2c[0:100, :], 0.0)
                        nc.sync.dma_start(raw.ap()[b, s2, :, :], ob2[0:100, :])

    nc.compile()
    return nc


def _host_prep(h_hat, pos_pickup, pos_delivery, solution, Wq1, Wk1, Wq2, Wk2, fc1_w):
    """Per-batch tiny maps G (128x32 each) + transposed/padded node features."""
    f32 = np.float32
    h_hat = np.asarray(h_hat, f32)
    pp = np.asarray(pos_pickup).astype(np.int64)
    pd = np.asarray(pos_delivery).astype(np.int64)
    sol = np.asarray(solution).astype(np.int64)
    Wq1 = np.asarray(Wq1, f32)
    Wk1 = np.asarray(Wk1, f32)
    Wq2 = np.asarray(Wq2, f32)
    Wk2 = np.asarray(Wk2, f32)
    fc1_w = np.asarray(fc1_w, f32)

    hhT = np.zeros((B, D, NP), f32)
    hnT = np.zeros((B, D, NP), f32)
    g1a = np.zeros((B, D, 32), f32)
    g2a = np.zeros((B, D, 32), f32)
    g1c = np.zeros((B, D, 128), f32)
    g2c = np.zeros((B, D, 128), f32)

    for b in range(B):
        hb = h_hat[b]  # (N, D)
        hnb = hb[sol[b]]  # (N, D) gathered neighbours
        hhT[b, :, :N] = hb.T
        hnT[b, :, :N] = hnb.T
        p = hb[pp[b]]  # (D,)
        dv = hb[pd[b]]
        # u[h] = Wk[h] @ (Wq[h]^T @ q): compat[n,h] = x[n] . u[h]
        U1p = np.stack([Wk1[h] @ (Wq1[h].T @ p) for h in range(H)], axis=1)
        U2p = np.stack([Wk2[h] @ (Wq2[h].T @ p) for h in range(H)], axis=1)
        U1d = np.stack([Wk1[h] @ (Wq1[h].T @ dv) for h in range(H)], axis=1)
        U2d = np.stack([Wk2[h] @ (Wq2[h].T @ dv) for h in range(H)], axis=1)
        g1a[b] = U1p @ fc1_w[0:4]  # h_hat -> A
        g2a[b] = U2p @ fc1_w[4:8]  # h_nb  -> A
        gc1 = U1d @ fc1_w[8:12]  # h_hat -> C
        gc2 = U2d @ fc1_w[12:16]  # h_nb  -> C
        g1c[b] = np.tile(gc1, (1, 4))
        g2c[b] = np.tile(gc2, (1, 4))
    return hhT, hnT, g1a, g2a, g1c, g2c


_last_results = None


def kernel(
    h_hat,
    pos_pickup,
    pos_delivery,
    solution,
    Wq1,
    Wk1,
    Wq2,
    Wk2,
    fc1_w,
    fc1_b,
    fc2_w,
    fc2_b,
    fc3_w,
    fc3_b,
):
    global _last_results
    from concourse.bass_utils import run_bass_kernel_spmd

    f32 = np.float32
    fc2_w = np.asarray(fc2_w, f32)
    fc1_b = np.asarray(fc1_b, f32)
    fc2_b = np.asarray(fc2_b, f32)
    fc3_w = np.asarray(fc3_w, f32)
    fc3_b = np.asarray(fc3_b, f32)

    hhT, hnT, g1a, g2a, g1c, g2c = _host_prep(
        h_hat, pos_pickup, pos_delivery, solution, Wq1, Wk1, Wq2, Wk2,
        np.asarray(fc1_w, f32),
    )

    # block-diagonal packed MLP weights (4 independent 32-blocks)
    w2d = np.zeros((D, 128), f32)
    w3d = np.zeros((D, 4), f32)
    for r in range(4):
        w2d[32 * r : 32 * r + 32, 32 * r : 32 * r + 32] = fc2_w
        w3d[32 * r : 32 * r + 32, r : r + 1] = fc3_w.reshape(32, 1)
    b1r = np.tile(fc1_b.reshape(32, 1), (4, 1)).astype(f32)
    b2r = np.tile(fc2_b.reshape(32, 1), (4, 1)).astype(f32)

    if "nc" not in _cache:
        _cache["nc"] = _build_program()
    nc = _cache["nc"]

    in_maps = []
    for c in range(NCORES):
        bs = slice(BPC * c, BPC * (c + 1))
        in_maps.append(
            {
                "hhT": np.ascontiguousarray(hhT[bs]).astype(np.float16),
                "hnT": np.ascontiguousarray(hnT[bs]).astype(np.float16),
                "g1a": np.ascontiguousarray(g1a[bs]).astype(np.float16),
                "g2a": np.ascontiguousarray(g2a[bs]).astype(np.float16),
                "g1c": np.ascontiguousarray(g1c[bs]).astype(np.float16),
                "g2c": np.ascontiguousarray(g2c[bs]).astype(np.float16),
                "w2d": w2d.astype(np.float16),
                "w3d": w3d.astype(np.float16),
                "b1r": b1r,
                "b2r": b2r,
            }
        )

    res = run_bass_kernel_spmd(nc, in_maps, core_ids=list(range(NCORES)))
    _last_results = res

    # un-permute: raw[b, s, 32v+r, 512h+j] holds out row 32s+16h+4v+r
    rows = np.arange(N)
    s_i = rows // 32
    rem = rows % 32
    h_i = rem // 16
    rem2 = rem % 16
    part = 32 * (rem2 // 4) + (rem2 % 4)
    foff = NP * h_i
    cols = np.arange(N)
    out = np.empty((B, N, N), f32)
    for c in range(NCORES):
        rawc = res.results[c]["raw"]  # [BPC, NSG, 100, 2*NP]
        for bb in range(BPC):
            out[BPC * c + bb] = rawc[bb, s_i[:, None], part[:, None], foff[:, None] + cols[None, :]]
    b3 = float(fc3_b.reshape(-1)[0])
    if b3 != 0.0:
        out = out + b3
    return out.astype(f32)


# revision 10
# speedup vs baseline: 4.1357x; 1.0456x over previous
"""Trainium2 Bass kernel for nn_NNSDecoder (gnn_message_passing).

Reference computation (B=16, N=501, D=128, H=4):
    out[b,i,j] = fc3 . relu(fc2^T relu(feat @ fc1 + b1) + b2) + b3
    feat[b,i,j] = [cp_pre[b,i], cp_post[b,i], cd_pre[b,j], cd_post[b,j]]  (4H=16)

Key algebra: compat[b,n,h] = x[b,n] . (Wk[h] Wq[h]^T q_b), so every
pickup/delivery-side term is linear in h_hat / h_nb rows.  Folding the
head projections and fc1 together gives per-batch 128x32 maps:
    A[b] = h_hat[b] @ G_A1 + h_nb[b] @ G_A2          (N x 32, row/i term)
    C[b] = h_hat[b] @ G_C1 + h_nb[b] @ G_C2          (N x 32, col/j term)
    out[b,i,j] = w3 . relu(W2^T relu(A[b,i] + C[b,j] + b1) + b2) + b3

The tiny G matrices (and the index gathers defining them) are computed on
host; the device does all O(N) and O(N^2) work.  Per 4-row i-tile t:
    X_t = relu(crep + a4[:,t])            (DVE tensor_scalar, f16, 2x mode)
    Z_t = W2blk @ X_t                     (PE matmul, block-diag f16)
    Y_t = relu(Z_t + b2)                  (ScalarE ACT / DVE, f16)
    po  = w3blk @ Y_t                     (PE matmul into packed PSUM)
i-tiles are processed in PAIRS sharing one 2-bank PSUM tile [128,1024]
(NP=512 = exact bank) so Y and the PSUM->SBUF copy run as single wide
ops; fc3 matmuls are emitted 2 pairs late (software pipelining) so the
PE never stalls waiting for Y; 8-tile supergroups drain with one wide
copy + 2 batched 16-row DMAs (partition-strided APs).

Sharding: batch dim 16 -> 8 cores x 2 batches (data parallel, weights
replicated). Full inputs in, full output out.
"""

import numpy as np

B, N, D, H = 16, 501, 128, 4
NCORES = 8
BPC = B // NCORES  # batches per core
NP = 512  # padded j: exact PSUM bank (512 f32 = 2KB)
NT = 126  # i-tiles of 4 rows (126*4 = 504 >= 501)
PAIRS = NT // 2  # 63 i-tile pairs per batch

# engine split knob: every Y2_DVE_EVERY-th pair's Y runs on DVE, rest ScalarE
Y2_DVE_EVERY = 1000000  # all Y2 on ScalarE: DVE-Y2 blocks X production (in-order queue)

_cache = {}


def _build_program():
    import concourse.bacc as bacc
    import concourse.mybir as mybir
    from concourse.tile import TileContext

    F32 = mybir.dt.float32
    F32R = mybir.dt.float32r
    F16 = mybir.dt.float16
    nc = bacc.Bacc("TRN2", target_bir_lowering=False, debug=False, num_devices=1)

    # packed inputs: fewer, larger DMAs (input load is DMA-queue-bound)
    hhd = nc.dram_tensor("hhd", [BPC, D, 2 * NP], F16, kind="ExternalInput")
    gpd = nc.dram_tensor("gpd", [BPC, D, 320], F16, kind="ExternalInput")
    wpd = nc.dram_tensor("wpd", [D, 132], F16, kind="ExternalInput")
    b1r = nc.dram_tensor("b1r", [D, 1], F32, kind="ExternalInput")
    b2r = nc.dram_tensor("b2r", [D, 1], F32, kind="ExternalInput")
    NSG = 16  # supergroups per batch (8 i-tiles / 32 rows each)
    raw = nc.dram_tensor("raw", [BPC, NSG, 100, 2 * NP], F16, kind="ExternalOutput")

    add = mybir.AluOpType.add
    amax = mybir.AluOpType.max
    Relu = mybir.ActivationFunctionType.Relu

    with TileContext(nc) as tc:
        with (
            tc.tile_pool(name="const", bufs=1) as cpool,
            tc.tile_pool(name="batch", bufs=2) as bpool,
            tc.tile_pool(name="x", bufs=8) as xpool,
            tc.tile_pool(name="y", bufs=6) as ypool,
            tc.tile_pool(name="o", bufs=3) as opool,
            tc.tile_pool(name="pz", bufs=2, space="PSUM") as pzpool,
            tc.tile_pool(name="po", bufs=2, space="PSUM") as popool,
        ):
            # input DMAs first (critical path), split across the two HWDGE
            # queues (SP + Activation); consts after.
            hhs = []
            gps = []
            for b in range(BPC):
                hh = bpool.tile([D, 2 * NP], F16, tag="hh")
                nc.sync.dma_start(hh[:], hhd.ap()[b, :, :])
                hhs.append(hh)
                gp = bpool.tile([D, 320], F16, tag="gp")
                nc.sync.dma_start(gp[:], gpd.ap()[b, :, :])
                gps.append(gp)
            wpt = cpool.tile([D, 132], F16)
            nc.sync.dma_start(wpt[:], wpd.ap()[:, :])
            w2t = wpt[:, 0:128]
            w3t = wpt[:, 128:132]
            b1t = cpool.tile([D, 1], F32)
            nc.sync.dma_start(b1t[:], b1r.ap()[:, :])
            b2t = cpool.tile([D, 1], F32)
            nc.sync.dma_start(b2t[:], b2r.ap()[:, :])

            # both batches' A/C setup up front so batch 1 starts instantly
            creps = []
            a4s = []
            for b in range(BPC):
                hh = hhs[b]
                gp = gps[b]
                pac = pzpool.tile([D, 2 * NP], F32, tag="pz")
                nc.tensor.matmul(
                    pac[:, 0:NP], gp[:, 64:192], hh[:, 0:NP], start=True, stop=False
                )
                nc.tensor.matmul(
                    pac[:, 0:NP], gp[:, 192:320], hh[:, NP : 2 * NP],
                    start=False, stop=True,
                )
                paa = pzpool.tile([D, 2 * NP], F32, tag="pz")
                nc.tensor.matmul(
                    paa[0:32, 0:NP], gp[:, 0:32], hh[:, 0:NP], start=True, stop=False
                )
                nc.tensor.matmul(
                    paa[0:32, 0:NP], gp[:, 32:64], hh[:, NP : 2 * NP],
                    start=False, stop=True,
                )

                # crep = C^T(rep4) + b1   (bias folded in)
                crep = bpool.tile([D, NP], F16, tag="crep")
                nc.vector.tensor_scalar_add(crep[:], pac[:, 0:NP], b1t[:, 0:1])
                creps.append(crep)

                # a4[32r+k, t] = A^T[k, 4t+r]  (i-tile column layout)
                a4 = bpool.tile([D, NT], F32, tag="a4")
                paa_r = paa[0:32, 0 : 4 * NT].rearrange("p (t r) -> p r t", r=4)
                for r in range(4):
                    nc.scalar.copy(a4[32 * r : 32 * r + 32, :], paa_r[:, r, :])
                a4s.append(a4)

            for b in range(BPC):
                crep = creps[b]
                a4 = a4s[b]
                # main pair loop, software-pipelined with per-stage skews so
                # no engine's in-order queue ever waits on another engine's
                # freshest output:
                #   iter i:  X,X for pair i (DVE)
                #            fc2 matmuls for pair i-1 (PE)
                #            Y2 wide op for pair i-2 (ACT / DVE)
                #            fc3 matmuls for pair i-3 (PE) + copy/DMA per group
                pend_x = {}
                pend_pz = {}
                pend_y = {}
                pend_po = {}
                po2 = None
                for p in range(PAIRS + 3 + 5):
                    if p < PAIRS:
                        t0 = 2 * p
                        xs = []
                        for t in (t0, t0 + 1):
                            x = xpool.tile([D, NP], F16, tag="x")
                            nc.vector.tensor_scalar(
                                out=x[:],
                                in0=crep[:],
                                scalar1=a4[:, t : t + 1],
                                scalar2=0.0,
                                op0=add,
                                op1=amax,
                            )
                            xs.append(x)
                        pend_x[p] = xs

                    pm = p - 1
                    if 0 <= pm < PAIRS:
                        xs = pend_x.pop(pm)
                        pz2 = pzpool.tile([D, 2 * NP], F32, tag="pz")
                        nc.tensor.matmul(
                            pz2[:, 0:NP], w2t[:], xs[0][:], start=True, stop=True
                        )
                        nc.tensor.matmul(
                            pz2[:, NP : 2 * NP], w2t[:], xs[1][:], start=True, stop=True
                        )
                        pend_pz[pm] = pz2

                    py = p - 2
                    if 0 <= py < PAIRS:
                        pz2 = pend_pz.pop(py)
                        y2 = ypool.tile([D, 2 * NP], F16, tag="y2")
                        if py % Y2_DVE_EVERY == Y2_DVE_EVERY - 1:
                            nc.vector.tensor_scalar(
                                out=y2[:],
                                in0=pz2[:],
                                scalar1=b2t[:, 0:1],
                                scalar2=0.0,
                                op0=add,
                                op1=amax,
                            )
                        else:
                            nc.scalar.activation(y2[:], pz2[:], Relu, bias=b2t[:, 0:1])
                        pend_y[py] = y2

                    pf = p - 3
                    if 0 <= pf < PAIRS:
                        s, q = divmod(pf, 4)
                        if q == 0:
                            po2 = popool.tile([D, 2 * NP], F32, tag="po")
                        y2 = pend_y.pop(pf)
                        for j in (0, 1):
                            st = 2 * q + j
                            h, v = st // 4, st % 4
                            nc.tensor.matmul(
                                po2[32 * v : 32 * v + 4, h * NP : h * NP + NP],
                                w3t[:],
                                y2[:, j * NP : j * NP + NP],
                                start=True,
                                stop=True,
                                tile_position=(0, 32 * v),
                            )
                        if q == 3 or pf == PAIRS - 1:
                            pend_po[s] = po2

                    # copy/DMA two iters after a supergroup's last fc3, so the
                    # DVE copy never waits mid-queue and never delays X ops
                    pc = p - 8
                    if pc >= 0 and pc % 4 == 0 and (pc // 4) in pend_po:
                        s2 = pc // 4
                        po2c = pend_po.pop(s2)
                        # used fc3 partitions are {32v+r, r<4} (v = slot%4,
                        # max 99); dump them raw, host un-permutes rows.
                        ob2 = opool.tile([D, 2 * NP], F16, tag="ob")
                        nc.vector.tensor_scalar_add(ob2[0:100, :], po2c[0:100, :], 0.0)
                        if b == BPC - 1 and s2 == NSG - 1:
                            # final dump: split across both HWDGE queues so the
                            # end-of-kernel drain halves
                            nc.sync.dma_start(
                                raw.ap()[b, s2, 0:50, :], ob2[0:50, :]
                            )
                            nc.sync.dma_start(
                                raw.ap()[b, s2, 50:100, :], ob2[50:100, :]
                            )
                        else:
                            nc.sync.dma_start(raw.ap()[b, s2, :, :], ob2[0:100, :])

    nc.compile()
    return nc


def _host_prep(h_hat, pos_pickup, pos_delivery, solution, Wq1, Wk1, Wq2, Wk2, fc1_w):
    """Per-batch tiny maps G (128x32 each) + transposed/padded node features."""
    f32 = np.float32
    h_hat = np.asarray(h_hat, f32)
    pp = np.asarray(pos_pickup).astype(np.int64)
    pd = np.asarray(pos_delivery).astype(np.int64)
    sol = np.asarray(solution).astype(np.int64)
    Wq1 = np.asarray(Wq1, f32)
    Wk1 = np.asarray(Wk1, f32)
    Wq2 = np.asarray(Wq2, f32)
    Wk2 = np.asarray(Wk2, f32)
    fc1_w = np.asarray(fc1_w, f32)

    hhT = np.zeros((B, D, NP), f32)
    hnT = np.zeros((B, D, NP), f32)
    g1a = np.zeros((B, D, 32), f32)
    g2a = np.zeros((B, D, 32), f32)
    g1c = np.zeros((B, D, 128), f32)
    g2c = np.zeros((B, D, 128), f32)

    for b in range(B):
        hb = h_hat[b]  # (N, D)
        hnb = hb[sol[b]]  # (N, D) gathered neighbours
        hhT[b, :, :N] = hb.T
        hnT[b, :, :N] = hnb.T
        p = hb[pp[b]]  # (D,)
        dv = hb[pd[b]]
        # u[h] = Wk[h] @ (Wq[h]^T @ q): compat[n,h] = x[n] . u[h]
        U1p = np.stack([Wk1[h] @ (Wq1[h].T @ p) for h in range(H)], axis=1)
        U2p = np.stack([Wk2[h] @ (Wq2[h].T @ p) for h in range(H)], axis=1)
        U1d = np.stack([Wk1[h] @ (Wq1[h].T @ dv) for h in range(H)], axis=1)
        U2d = np.stack([Wk2[h] @ (Wq2[h].T @ dv) for h in range(H)], axis=1)
        g1a[b] = U1p @ fc1_w[0:4]  # h_hat -> A
        g2a[b] = U2p @ fc1_w[4:8]  # h_nb  -> A
        gc1 = U1d @ fc1_w[8:12]  # h_hat -> C
        gc2 = U2d @ fc1_w[12:16]  # h_nb  -> C
        g1c[b] = np.tile(gc1, (1, 4))
        g2c[b] = np.tile(gc2, (1, 4))
    return hhT, hnT, g1a, g2a, g1c, g2c


_last_results = None


def kernel(
    h_hat,
    pos_pickup,
    pos_delivery,
    solution,
    Wq1,
    Wk1,
    Wq2,
    Wk2,
    fc1_w,
    fc1_b,
    fc2_w,
    fc2_b,
    fc3_w,
    fc3_b,
):
    global _last_results
    from concourse.bass_utils import run_bass_kernel_spmd

    f32 = np.float32
    fc2_w = np.asarray(fc2_w, f32)
    fc1_b = np.asarray(fc1_b, f32)
    fc2_b = np.asarray(fc2_b, f32)
    fc3_w = np.asarray(fc3_w, f32)
    fc3_b = np.asarray(fc3_b, f32)

    hhT, hnT, g1a, g2a, g1c, g2c = _host_prep(
        h_hat, pos_pickup, pos_delivery, solution, Wq1, Wk1, Wq2, Wk2,
        np.asarray(fc1_w, f32),
    )

    # block-diagonal packed MLP weights (4 independent 32-blocks)
    w2d = np.zeros((D, 128), f32)
    w3d = np.zeros((D, 4), f32)
    for r in range(4):
        w2d[32 * r : 32 * r + 32, 32 * r : 32 * r + 32] = fc2_w
        w3d[32 * r : 32 * r + 32, r : r + 1] = fc3_w.reshape(32, 1)
    b1r = np.tile(fc1_b.reshape(32, 1), (4, 1)).astype(f32)
    b2r = np.tile(fc2_b.reshape(32, 1), (4, 1)).astype(f32)

    if "nc" not in _cache:
        _cache["nc"] = _build_program()
    nc = _cache["nc"]

    hhp = np.concatenate([hhT, hnT], axis=2).astype(np.float16)  # [B, D, 2*NP]
    gp = np.concatenate([g1a, g2a, g1c, g2c], axis=2).astype(np.float16)  # [B, D, 320]
    wp = np.concatenate([w2d, w3d], axis=1).astype(np.float16)  # [D, 132]
    in_maps = []
    for c in range(NCORES):
        bs = slice(BPC * c, BPC * (c + 1))
        in_maps.append(
            {
                "hhd": np.ascontiguousarray(hhp[bs]),
                "gpd": np.ascontiguousarray(gp[bs]),
                "wpd": wp,
                "b1r": b1r,
                "b2r": b2r,
            }
        )

    res = run_bass_kernel_spmd(nc, in_maps, core_ids=list(range(NCORES)))
    _last_results = res

    # un-permute: raw[b, s, 32v+r, 512h+j] holds out row 32s+16h+4v+r
    rows = np.arange(N)
    s_i = rows // 32
    rem = rows % 32
    h_i = rem // 16
    rem2 = rem % 16
    part = 32 * (rem2 // 4) + (rem2 % 4)
    foff = NP * h_i
    cols = np.arange(N)
    out = np.empty((B, N, N), f32)
    for c in range(NCORES):
        rawc = res.results[c]["raw"].astype(f32)  # [BPC, NSG, 100, 2*NP]
        for bb in range(BPC):
            out[BPC * c + bb] = rawc[bb, s_i[:, None], part[:, None], foff[:, None] + cols[None, :]]
    b3 = float(fc3_b.reshape(-1)[0])
    if b3 != 0.0:
        out = out + b3
    return out.astype(f32)


# revision 12
# speedup vs baseline: 4.2725x; 1.0331x over previous
"""Trainium2 Bass kernel for nn_NNSDecoder (gnn_message_passing).

Reference computation (B=16, N=501, D=128, H=4):
    out[b,i,j] = fc3 . relu(fc2^T relu(feat @ fc1 + b1) + b2) + b3
    feat[b,i,j] = [cp_pre[b,i], cp_post[b,i], cd_pre[b,j], cd_post[b,j]]  (4H=16)

Key algebra: compat[b,n,h] = x[b,n] . (Wk[h] Wq[h]^T q_b), so every
pickup/delivery-side term is linear in h_hat / h_nb rows.  Folding the
head projections and fc1 together gives per-batch maps
    A[b] = h_hat[b] @ G_A1 + h_nb[b] @ G_A2          (N x 32, row/i term)
    C[b] = h_hat[b] @ G_C1 + h_nb[b] @ G_C2          (N x 32, col/j term)
    out[b,i,j] = w3 . relu(W2^T relu(A[b,i] + C[b,j] + b1) + b2) + b3
A and C are tiny (N x 32) and are computed on HOST in fp32; the device
receives crep = (C+b1) replicated 4x across partitions (f16) and
a4[32r+k, t] = A[4t+r, k] (f32), and does only the O(N^2) work.

Per 4-row i-tile t:
    X_t = relu(crep + a4[:,t])            (DVE tensor_scalar, f16, 2x mode)
    Z_t = W2blk @ X_t                     (PE matmul, block-diag f16)
    Y_t = relu(Z_t + b2)                  (ScalarE ACT, f16)
    po  = w3blk @ Y_t                     (PE matmul into packed PSUM)
i-tiles are processed in PAIRS sharing one 2-bank PSUM tile [128,1024]
(NP=512 = exact bank) so Y runs as a single wide op; the stages are
software-pipelined with per-stage skews (X@i, fc2@i-1, Y@i-2, fc3@i-3,
PSUM->SBUF copy@deep skew on DVE) so no engine's in-order queue waits
on another engine's freshest output.  8-tile supergroups drain with one
wide f16 copy + one contiguous raw-dump DMA; the host un-permutes rows.

Sharding: batch dim 16 -> 8 cores x 2 batches (data parallel, weights
replicated). Full inputs in, full output out.
"""

import numpy as np

B, N, D, H = 16, 501, 128, 4
NCORES = 8
BPC = B // NCORES  # batches per core
NP = 512  # padded j: exact PSUM bank (512 f32 = 2KB)
NT = 126  # i-tiles of 4 rows (126*4 = 504 >= 501)
PAIRS = NT // 2  # 63 i-tile pairs per batch
NSG = 16  # supergroups per batch (8 i-tiles / 32 output rows each)

_cache = {}


def _build_program():
    import concourse.bacc as bacc
    import concourse.mybir as mybir
    from concourse.tile import TileContext

    F32 = mybir.dt.float32
    F16 = mybir.dt.float16
    nc = bacc.Bacc("TRN2", target_bir_lowering=False, debug=False, num_devices=1)

    crd = nc.dram_tensor("crd", [BPC, D, NP], F16, kind="ExternalInput")
    a4d = nc.dram_tensor("a4d", [BPC, D, NT], F32, kind="ExternalInput")
    wpd = nc.dram_tensor("wpd", [D, 132], F16, kind="ExternalInput")
    b2r = nc.dram_tensor("b2r", [D, 1], F32, kind="ExternalInput")
    raw = nc.dram_tensor("raw", [BPC, NSG, 100, 2 * NP], F16, kind="ExternalOutput")

    add = mybir.AluOpType.add
    amax = mybir.AluOpType.max
    Relu = mybir.ActivationFunctionType.Relu

    with TileContext(nc) as tc:
        with (
            tc.tile_pool(name="const", bufs=1) as cpool,
            tc.tile_pool(name="batch", bufs=2) as bpool,
            tc.tile_pool(name="x", bufs=8) as xpool,
            tc.tile_pool(name="y", bufs=6) as ypool,
            tc.tile_pool(name="o", bufs=3) as opool,
            tc.tile_pool(name="pz", bufs=2, space="PSUM") as pzpool,
            tc.tile_pool(name="po", bufs=2, space="PSUM") as popool,
        ):
            # inputs for batch 0 first: they gate the first X / fc2
            creps = []
            a4s = []
            for b in range(BPC):
                crep = bpool.tile([D, NP], F16, tag="crep")
                nc.sync.dma_start(crep[:], crd.ap()[b, :, :])
                creps.append(crep)
                a4 = bpool.tile([D, NT], F32, tag="a4")
                nc.sync.dma_start(a4[:], a4d.ap()[b, :, :])
                a4s.append(a4)
                if b == 0:
                    wpt = cpool.tile([D, 132], F16)
                    nc.sync.dma_start(wpt[:], wpd.ap()[:, :])
                    w2t = wpt[:, 0:128]
                    w3t = wpt[:, 128:132]
                    b2t = cpool.tile([D, 1], F32)
                    nc.sync.dma_start(b2t[:], b2r.ap()[:, :])

            for b in range(BPC):
                crep = creps[b]
                a4 = a4s[b]

                # main pair loop, software-pipelined with per-stage skews
                pend_x = {}
                pend_pz = {}
                pend_y = {}
                pend_po = {}
                po2 = None
                for p in range(PAIRS + 3 + 5):
                    if p < PAIRS:
                        t0 = 2 * p
                        xs = []
                        for t in (t0, t0 + 1):
                            x = xpool.tile([D, NP], F16, tag="x")
                            nc.vector.tensor_scalar(
                                out=x[:],
                                in0=crep[:],
                                scalar1=a4[:, t : t + 1],
                                scalar2=0.0,
                                op0=add,
                                op1=amax,
                            )
                            xs.append(x)
                        pend_x[p] = xs

                    pm = p - 1
                    if 0 <= pm < PAIRS:
                        xs = pend_x.pop(pm)
                        pz2 = pzpool.tile([D, 2 * NP], F32, tag="pz")
                        nc.tensor.matmul(
                            pz2[:, 0:NP], w2t, xs[0][:], start=True, stop=True
                        )
                        nc.tensor.matmul(
                            pz2[:, NP : 2 * NP], w2t, xs[1][:], start=True, stop=True
                        )
                        pend_pz[pm] = pz2

                    py = p - 2
                    if 0 <= py < PAIRS:
                        pz2 = pend_pz.pop(py)
                        y2 = ypool.tile([D, 2 * NP], F16, tag="y2")
                        nc.scalar.activation(y2[:], pz2[:], Relu, bias=b2t[:, 0:1])
                        pend_y[py] = y2

                    pf = p - 3
                    if 0 <= pf < PAIRS:
                        s, q = divmod(pf, 4)
                        if q == 0:
                            po2 = popool.tile([D, 2 * NP], F32, tag="po")
                        y2 = pend_y.pop(pf)
                        for j in (0, 1):
                            st = 2 * q + j
                            h, v = st // 4, st % 4
                            nc.tensor.matmul(
                                po2[32 * v : 32 * v + 4, h * NP : h * NP + NP],
                                w3t,
                                y2[:, j * NP : j * NP + NP],
                                start=True,
                                stop=True,
                                tile_position=(0, 32 * v),
                            )
                        if pf == PAIRS - 1 and b == BPC - 1:
                            # very last supergroup: shallow copy + trimmed
                            # split dump so the end-of-kernel drain is short
                            ob2 = opool.tile([D, 2 * NP], F16, tag="ob")
                            nc.vector.tensor_scalar_add(
                                ob2[0:100, :], po2[0:100, :], 0.0
                            )
                            nc.sync.dma_start(
                                raw.ap()[b, s, 0:100, 0:NP], ob2[0:100, 0:NP]
                            )
                            nc.sync.dma_start(
                                raw.ap()[b, s, 0:40, NP : 2 * NP],
                                ob2[0:40, NP : 2 * NP],
                            )
                        elif q == 3 or pf == PAIRS - 1:
                            pend_po[s] = po2

                    # copy/DMA two iters after a supergroup's last fc3, so the
                    # DVE copy never waits mid-queue and never delays X ops
                    pc = p - 8
                    if pc >= 0 and pc % 4 == 0 and (pc // 4) in pend_po:
                        s2 = pc // 4
                        po2c = pend_po.pop(s2)
                        # used fc3 partitions are {32v+r, r<4} (v = slot%4,
                        # max 99); dump them raw, host un-permutes rows.
                        ob2 = opool.tile([D, 2 * NP], F16, tag="ob")
                        nc.vector.tensor_scalar_add(ob2[0:100, :], po2c[0:100, :], 0.0)
                        nc.sync.dma_start(raw.ap()[b, s2, :, :], ob2[0:100, :])

    nc.compile()
    return nc


def _host_prep(h_hat, pos_pickup, pos_delivery, solution, Wq1, Wk1, Wq2, Wk2,
               fc1_w, fc1_b):
    """Host-side A/C maps folded with fc1: returns crep (f16) and a4 (f32)."""
    f32 = np.float32
    h_hat = np.asarray(h_hat, f32)
    pp = np.asarray(pos_pickup).astype(np.int64)
    pd = np.asarray(pos_delivery).astype(np.int64)
    sol = np.asarray(solution).astype(np.int64)
    Wq1 = np.asarray(Wq1, f32)
    Wk1 = np.asarray(Wk1, f32)
    Wq2 = np.asarray(Wq2, f32)
    Wk2 = np.asarray(Wk2, f32)
    fc1_w = np.asarray(fc1_w, f32)
    fc1_b = np.asarray(fc1_b, f32)

    crep = np.zeros((B, D, NP), np.float16)
    a4 = np.zeros((B, D, NT), f32)

    for b in range(B):
        hb = h_hat[b]  # (N, D)
        hnb = hb[sol[b]]  # (N, D) gathered neighbours
        p = hb[pp[b]]  # (D,)
        dv = hb[pd[b]]
        # u[h] = Wk[h] @ (Wq[h]^T @ q): compat[n,h] = x[n] . u[h]
        U1p = np.stack([Wk1[h] @ (Wq1[h].T @ p) for h in range(H)], axis=1)
        U2p = np.stack([Wk2[h] @ (Wq2[h].T @ p) for h in range(H)], axis=1)
        U1d = np.stack([Wk1[h] @ (Wq1[h].T @ dv) for h in range(H)], axis=1)
        U2d = np.stack([Wk2[h] @ (Wq2[h].T @ dv) for h in range(H)], axis=1)
        A = hb @ (U1p @ fc1_w[0:4]) + hnb @ (U2p @ fc1_w[4:8])  # (N, 32)
        C = hb @ (U1d @ fc1_w[8:12]) + hnb @ (U2d @ fc1_w[12:16])  # (N, 32)
        Cp = np.zeros((NP, 32), f32)
        Cp[:N] = C
        crep[b] = np.tile((Cp + fc1_b).T.astype(np.float16), (4, 1))
        Ap = np.zeros((4 * NT, 32), f32)
        Ap[:N] = A
        # a4[32r+k, t] = A[4t+r, k]
        a4[b] = Ap.reshape(NT, 4, 32).transpose(1, 2, 0).reshape(D, NT)
    return crep, a4


_last_results = None


def kernel(
    h_hat,
    pos_pickup,
    pos_delivery,
    solution,
    Wq1,
    Wk1,
    Wq2,
    Wk2,
    fc1_w,
    fc1_b,
    fc2_w,
    fc2_b,
    fc3_w,
    fc3_b,
):
    global _last_results
    from concourse.bass_utils import run_bass_kernel_spmd

    f32 = np.float32
    fc2_w = np.asarray(fc2_w, f32)
    fc2_b = np.asarray(fc2_b, f32)
    fc3_w = np.asarray(fc3_w, f32)
    fc3_b = np.asarray(fc3_b, f32)

    crep, a4 = _host_prep(
        h_hat, pos_pickup, pos_delivery, solution, Wq1, Wk1, Wq2, Wk2,
        np.asarray(fc1_w, f32), np.asarray(fc1_b, f32),
    )

    # block-diagonal packed MLP weights (4 independent 32-blocks)
    w2d = np.zeros((D, 128), f32)
    w3d = np.zeros((D, 4), f32)
    for r in range(4):
        w2d[32 * r : 32 * r + 32, 32 * r : 32 * r + 32] = fc2_w
        w3d[32 * r : 32 * r + 32, r : r + 1] = fc3_w.reshape(32, 1)
    b2r = np.tile(fc2_b.reshape(32, 1), (4, 1)).astype(f32)
    wp = np.concatenate([w2d, w3d], axis=1).astype(np.float16)  # [D, 132]

    if "nc" not in _cache:
        _cache["nc"] = _build_program()
    nc = _cache["nc"]

    in_maps = []
    for c in range(NCORES):
        bs = slice(BPC * c, BPC * (c + 1))
        in_maps.append(
            {
                "crd": np.ascontiguousarray(crep[bs]),
                "a4d": np.ascontiguousarray(a4[bs]),
                "wpd": wp,
                "b2r": b2r,
            }
        )

    res = run_bass_kernel_spmd(nc, in_maps, core_ids=list(range(NCORES)))
    _last_results = res

    # un-permute: raw[b, s, 32v+r, 512h+j] holds out row 32s+16h+4v+r
    rows = np.arange(N)
    s_i = rows // 32
    rem = rows % 32
    h_i = rem // 16
    rem2 = rem % 16
    part = 32 * (rem2 // 4) + (rem2 % 4)
    foff = NP * h_i
    cols = np.arange(N)
    out = np.empty((B, N, N), f32)
    for c in range(NCORES):
        rawc = res.results[c]["raw"].astype(f32)  # [BPC, NSG, 100, 2*NP]
        for bb in range(BPC):
            out[BPC * c + bb] = rawc[
                bb, s_i[:, None], part[:, None], foff[:, None] + cols[None, :]
            ]
    b3 = float(fc3_b.reshape(-1)[0])
    if b3 != 0.0:
        out = out + b3
    return out.astype(f32)
